# revision 1
# baseline (speedup 1.0000x reference)
"""Trainium2 Bass kernel for the plane-sweep ZNCC photometric loss module.

Contract: kernel(**inputs) takes the FULL unsharded inputs and returns the
full output (a scalar float32 loss).  Internally the (H, W) pixel grid is
sharded across 8 NeuronCores along H (180 rows per core).  Each core
computes surface normals, the per-pixel homography warp, bilinear sampling
of the target images (via a window-gather scheme), windowed 7x7 ZNCC, and
partial (score, count) sums; the host adds the 16 partial scalars and
finishes the loss.
"""

import numpy as np

H_IMG, W_IMG = 1440, 1920
NCORES = 8
SHARD = H_IMG // NCORES          # 180 rows per core

# chunk geometry (per core): 2 row-chunks x 2 col-chunks
R0S = [0, 110]                   # tile-row offsets inside the shard band
C0S = [0, 960]                   # absolute col origins
TCOL = 972                       # tile cols = 960 + 12 halo
NY = 3                           # y-combo count (window row candidates)
WROWS = 144                      # window rows per chunk (128 + NY + margins)
WCOLS = 980                      # window cols (972 + x-margin)
JW = 3                           # window col of output col q at dx=0
PY, PX = 160, 96                 # zero padding around targets
BLK = 8                          # shear block (cols)
EPS = 1e-8

_cache = {}


def _rodrigues(v):
    th = np.linalg.norm(v) + 1e-12
    k = v / th
    Kx = np.array([[0, -k[2], k[1]], [k[2], 0, -k[0]], [-k[1], k[0], 0]])
    return np.eye(3) + np.sin(th) * Kx + (1 - np.cos(th)) * (Kx @ Kx)


def _build_program():
    """Build the (input-independent) SPMD bass program."""
    import concourse.bacc as bacc
    import concourse.mybir as mybir
    from concourse import tile

    f32 = mybir.dt.float32
    bf16 = mybir.dt.bfloat16
    AL = mybir.AluOpType

    nc = bacc.Bacc(None, target_bir_lowering=False)

    # ---- DRAM inputs (per core) ----
    d_depth = nc.dram_tensor("depth_band", [240, 1936], f32, kind="ExternalInput")
    d_ref = nc.dram_tensor("ref_band", [238, 1932], f32, kind="ExternalInput")
    d_win = nc.dram_tensor("win", [2, 2, 2, WROWS, WCOLS], f32, kind="ExternalInput")
    d_ytab = nc.dram_tensor("ytab", [2, 2, 2, NY + 1, 128], f32, kind="ExternalInput")
    d_yshm = nc.dram_tensor("yshm", [2, 2, 2, 128, TCOL], f32, kind="ExternalInput")
    d_xjm = nc.dram_tensor("xjm", [2, 2, 2, 128, TCOL], f32, kind="ExternalInput")
    d_jabs = nc.dram_tensor("jabs", [2, 128, TCOL], f32, kind="ExternalInput")
    d_cxm = nc.dram_tensor("cxm", [2, 128, TCOL], f32, kind="ExternalInput")
    d_cym = nc.dram_tensor("cym", [2, 128], f32, kind="ExternalInput")
    d_sc = nc.dram_tensor("sc", [32, 128], f32, kind="ExternalInput")
    # sc rows: 0: fsq=(f/2)^2, 1..6: b[t][i] (t-major), 7..12: M[t][i,0],
    # 13..18: Mrow-bias ryv?? -> see host; 19,20: rowvalid per rc is separate
    d_ryv = nc.dram_tensor("ryv", [2, 3, 2, 128], f32, kind="ExternalInput")
    d_rowv = nc.dram_tensor("rowv", [2, 128], f32, kind="ExternalInput")
    d_zmm = nc.dram_tensor("zmm", [2, 128], f32, kind="ExternalInput")

    d_band = nc.dram_tensor("band7", [128, 128], f32, kind="ExternalInput")
    d_rowofp = nc.dram_tensor("rowofp", [2, 2, 2, 128], f32, kind="ExternalInput")

    d_acc = nc.dram_tensor("acc", [128, 4], f32, kind="ExternalOutput")

    from contextlib import ExitStack
    ctx = ExitStack()
    with tile.TileContext(nc) as tc:
        with tc.tile_pool(name="io", bufs=1) as iop, \
             tc.tile_pool(name="wk", bufs=1) as wk, \
             tc.tile_pool(name="gat", bufs=2) as gp, \
             tc.tile_pool(name="ps", bufs=2, space="PSUM") as psp:

            acc = iop.tile([128, 4], f32)
            nc.vector.memset(acc[:], 0.0)

            jabs_t = [iop.tile([128, TCOL], f32, tag=f"jabs{c}", name=f"jabs{c}") for c in range(2)]
            cxm_t = [iop.tile([128, TCOL], f32, tag=f"cxm{c}", name=f"cxm{c}") for c in range(2)]
            for c in range(2):
                nc.sync.dma_start(jabs_t[c][:], d_jabs[c])
                nc.sync.dma_start(cxm_t[c][:], d_cxm[c])
            cym_t = iop.tile([128, 2], f32)
            nc.sync.dma_start(cym_t[:], d_cym.rearrange("r p -> p r"))
            sc_t = iop.tile([128, 32], f32)
            nc.sync.dma_start(sc_t[:], d_sc.rearrange("r p -> p r"))
            ryv_t = iop.tile([128, 12], f32)
            nc.sync.dma_start(ryv_t[:], d_ryv.rearrange("t i r p -> p (t i r)"))
            rowv_t = iop.tile([128, 2], f32)
            nc.sync.dma_start(rowv_t[:], d_rowv.rearrange("r p -> p r"))
            zmm_t = iop.tile([128, 2], f32)
            nc.sync.dma_start(zmm_t[:], d_zmm.rearrange("r p -> p r"))
            band_t = iop.tile([128, 128], f32)
            nc.sync.dma_start(band_t[:], d_band[:])

            for rc in range(2):
                for cc in range(2):
                    R0 = R0S[rc]
                    # ---------- load depth/ref ----------
                    dptA = wk.tile([128, 974], f32, tag="dptA", name="dptA", bufs=2)
                    nc.sync.dma_start(dptA[:], d_depth[R0:R0 + 128, C0S[cc]:C0S[cc] + 974])
                    dptB = wk.tile([2, 974], f32, tag="dptB", name="dptB")
                    nc.sync.dma_start(dptB[:], d_depth[R0 + 128:R0 + 130, C0S[cc]:C0S[cc] + 974])
                    dsh = wk.tile([128, 974], f32, tag="dsh", name="dsh")
                    nc.sync.dma_start(dsh[0:126, :], dptA[2:128, :])
                    nc.sync.dma_start(dsh[126:128, :], dptB[0:2, :])
                    rt = wk.tile([128, TCOL], f32, tag="rt", name="rt", bufs=2)
                    nc.sync.dma_start(rt[:], d_ref[R0:R0 + 128, C0S[cc]:C0S[cc] + TCOL])

                    # ---------- s field (shared by both targets) ----------
                    # du2 = d(X+1)-d(X-1), dv2 = d(Y+1)-d(Y-1)   [128, TCOL]
                    du2 = wk.tile([128, TCOL], f32, tag="du2", name="du2")
                    nc.vector.tensor_tensor(out=du2[:], in0=dptA[:, 2:974], in1=dptA[:, 0:TCOL], op=AL.subtract)
                    dv2 = wk.tile([128, TCOL], f32, tag="dv2", name="dv2")
                    nc.vector.tensor_tensor(out=dv2[:], in0=dsh[:, 1:973], in1=dptA[:, 1:973], op=AL.subtract)
                    dep = dptA[:, 1:973]  # depth at the pixel
                    # nz2 = cxm*du2 + cym*dv2  (2x the reference nz + 2d)
                    t1 = wk.tile([128, TCOL], f32, tag="t1", name="t1")
                    nc.vector.tensor_tensor(out=t1[:], in0=cxm_t[cc][:], in1=du2[:], op=AL.mult)
                    nc.vector.scalar_tensor_tensor(out=t1[:], in0=dv2[:], scalar=cym_t[:, rc:rc + 1], in1=t1[:], op0=AL.mult, op1=AL.add)
                    # nz = 0.5*t1 - dep
                    nz = wk.tile([128, TCOL], f32, tag="nz", name="nz")
                    nc.vector.scalar_tensor_tensor(out=nz[:], in0=t1[:], scalar=0.5, in1=dep, op0=AL.mult, op1=AL.subtract)
                    # n2 = nz^2 + fsq*(du2^2 + dv2^2)
                    n2 = wk.tile([128, TCOL], f32, tag="n2", name="n2")
                    nc.vector.tensor_tensor(out=n2[:], in0=du2[:], in1=du2[:], op=AL.mult)
                    t2 = wk.tile([128, TCOL], f32, tag="t2", name="t2")
                    nc.vector.tensor_tensor(out=t2[:], in0=dv2[:], in1=dv2[:], op=AL.mult)
                    nc.vector.tensor_tensor(out=n2[:], in0=n2[:], in1=t2[:], op=AL.add)
                    nc.vector.scalar_tensor_tensor(out=t2[:], in0=nz[:], scalar=1.0, in1=nz[:], op0=AL.mult, op1=AL.mult)
                    nc.vector.scalar_tensor_tensor(out=n2[:], in0=n2[:], scalar=sc_t[:, 0:1], in1=t2[:], op0=AL.mult, op1=AL.add)
                    # nrm = sqrt(n2); den = (nrm+eps)*(dep+eps); s = (0.25*(du2+dv2)-dep)/den
                    nrm = t2
                    nc.scalar.activation(nrm[:], n2[:], mybir.ActivationFunctionType.Sqrt)
                    den = wk.tile([128, TCOL], f32, tag="den", name="den")
                    nc.vector.tensor_scalar(out=den[:], in0=dep, scalar1=EPS, scalar2=None, op0=AL.add)
                    nc.vector.tensor_scalar(out=nrm[:], in0=nrm[:], scalar1=EPS, scalar2=None, op0=AL.add)
                    nc.vector.tensor_tensor(out=den[:], in0=den[:], in1=nrm[:], op=AL.mult)
                    nc.vector.reciprocal_approx_fast(out=den[:], in_=den[:])
                    sfield = wk.tile([128, TCOL], f32, tag="sfield", name="sfield")
                    nc.vector.tensor_tensor(out=sfield[:], in0=du2[:], in1=dv2[:], op=AL.add)
                    nc.vector.scalar_tensor_tensor(out=sfield[:], in0=sfield[:], scalar=0.25, in1=dep, op0=AL.mult, op1=AL.subtract)
                    nc.vector.tensor_tensor(out=sfield[:], in0=sfield[:], in1=den[:], op=AL.mult)

                    # shared ref box terms for this chunk
                    boxr = wk.tile([128, TCOL], f32, tag="boxr", name="boxr")
                    _box7(nc, tc, wk, boxr, rt, f32, AL, band_t, psp, ctx)
                    rcen = wk.tile([128, TCOL], f32, tag="rcen", name="rcen")
                    nc.vector.tensor_tensor(out=rcen[:], in0=rt[:], in1=boxr[:], op=AL.subtract)
                    rc2 = wk.tile([128, TCOL], f32, tag="prod", name="rc2")
                    nc.vector.tensor_tensor(out=rc2[:], in0=rcen[:], in1=rcen[:], op=AL.mult)
                    boxrr = wk.tile([128, TCOL], f32, tag="boxrr", name="boxrr")
                    _box7(nc, tc, wk, boxrr, rc2, f32, AL, band_t, psp, ctx)

                    for t in range(2):
                        # ---------- warp ----------
                        ww = []
                        for i in range(3):
                            w_ = wk.tile([128, TCOL], f32, tag=["du2","dv2","nz"][i], name=f"w{i}")
                            # w = M[i,0]*jabs + ryv'  (ryv' = M[i,1]*y + M[i,2])
                            nc.vector.tensor_scalar(
                                out=w_[:], in0=jabs_t[cc][:],
                                scalar1=sc_t[:, 7 + t * 3 + i:8 + t * 3 + i],
                                scalar2=ryv_t[:, t * 6 + i * 2 + rc:t * 6 + i * 2 + rc + 1],
                                op0=AL.mult, op1=AL.add)
                            # += b[i]*s
                            nc.vector.scalar_tensor_tensor(
                                out=w_[:], in0=sfield[:],
                                scalar=sc_t[:, 1 + t * 3 + i:2 + t * 3 + i],
                                in1=w_[:], op0=AL.mult, op1=AL.add)
                            ww.append(w_)
                        rec = wk.tile([128, TCOL], f32, tag="n2", name="rec")
                        nc.vector.tensor_scalar(out=rec[:], in0=ww[2][:], scalar1=EPS, scalar2=1e-6, op0=AL.add, op1=AL.max)
                        nc.vector.reciprocal_approx_fast(out=rec[:], in_=rec[:])
                        wx = wk.tile([128, TCOL], f32, tag="wx", name="wx")
                        nc.vector.tensor_tensor(out=wx[:], in0=ww[0][:], in1=rec[:], op=AL.mult)
                        wy = wk.tile([128, TCOL], f32, tag="wy", name="wy")
                        nc.vector.tensor_tensor(out=wy[:], in0=ww[1][:], in1=rec[:], op=AL.mult)

                        # oob mask: big = max(-1-wx, wx-W, -1-wy, wy-H) > 0
                        oob = wk.tile([128, TCOL], f32, tag="oob", name="oob")
                        nc.vector.tensor_scalar(out=oob[:], in0=wx[:], scalar1=-1.0, scalar2=1.0, op0=AL.mult, op1=AL.subtract)
                        nc.vector.tensor_scalar(out=t1[:], in0=wx[:], scalar1=float(W_IMG), scalar2=None, op0=AL.subtract)
                        nc.vector.tensor_tensor(out=oob[:], in0=oob[:], in1=t1[:], op=AL.max)
                        nc.vector.tensor_scalar(out=t1[:], in0=wy[:], scalar1=-1.0, scalar2=1.0, op0=AL.mult, op1=AL.subtract)
                        nc.vector.tensor_tensor(out=oob[:], in0=oob[:], in1=t1[:], op=AL.max)
                        nc.vector.tensor_scalar(out=t1[:], in0=wy[:], scalar1=float(H_IMG), scalar2=None, op0=AL.subtract)
                        nc.vector.tensor_tensor(out=oob[:], in0=oob[:], in1=t1[:], op=AL.max)
                        # oob > 0 means the pixel samples fully outside the image

                        # clamp wx, wy to padded-safe range and floor via mod
                        nc.vector.tensor_scalar(out=wx[:], in0=wx[:], scalar1=-4.0, scalar2=float(W_IMG + 3), op0=AL.max, op1=AL.min)
                        nc.vector.tensor_scalar(out=wy[:], in0=wy[:], scalar1=-4.0, scalar2=float(H_IMG + 3), op0=AL.max, op1=AL.min)
                        nc.vector.tensor_scalar(out=wx[:], in0=wx[:], scalar1=8.0, scalar2=None, op0=AL.add)
                        nc.vector.tensor_scalar(out=wy[:], in0=wy[:], scalar1=8.0, scalar2=None, op0=AL.add)
                        iti = wk.tile([128, TCOL], mybir.dt.int32, tag="iti", name="iti")
                        fx = wk.tile([128, TCOL], f32, tag="fx", name="fx")
                        fy = wk.tile([128, TCOL], f32, tag="fy", name="fy")
                        flo = wk.tile([128, TCOL], f32, tag="flo", name="flo")
                        # floor via round-nearest cast of (x - 0.5); exact-tie cases are
                        # measure-zero and land on clamped/zeroed pixels
                        nc.vector.tensor_scalar(out=flo[:], in0=wx[:], scalar1=0.5, scalar2=None, op0=AL.subtract)
                        nc.vector.tensor_copy(out=iti[:], in_=flo[:])
                        nc.vector.tensor_copy(out=flo[:], in_=iti[:])
                        nc.vector.tensor_tensor(out=fx[:], in0=wx[:], in1=flo[:], op=AL.subtract)
                        x0f = wx  # x0f = floor - 8
                        nc.vector.tensor_scalar(out=x0f[:], in0=flo[:], scalar1=8.0, scalar2=None, op0=AL.subtract)
                        nc.vector.tensor_scalar(out=fy[:], in0=wy[:], scalar1=0.5, scalar2=None, op0=AL.subtract)
                        nc.vector.tensor_copy(out=iti[:], in_=fy[:])
                        nc.vector.tensor_copy(out=flo[:], in_=iti[:])
                        nc.vector.tensor_tensor(out=fy[:], in0=wy[:], in1=flo[:], op=AL.subtract)
                        y0f = wy
                        nc.vector.tensor_scalar(out=y0f[:], in0=flo[:], scalar1=8.0, scalar2=None, op0=AL.subtract)

                        # ---------- gather ----------
                        # load window + variants
                        w0 = gp.tile([128, WCOLS], bf16, tag="w0", name="w0", bufs=2)
                        nc.gpsimd.dma_start(w0[:], d_win[t, rc, cc, 0:128, :])
                        w1 = gp.tile([16, WCOLS], bf16, tag="w1", name="w1", bufs=2)
                        nc.gpsimd.dma_start(w1[:], d_win[t, rc, cc, 128:WROWS, :])
                        ytab_t = gp.tile([128, NY + 1], f32, tag="ytab", name="ytab")
                        nc.sync.dma_start(ytab_t[:], d_ytab[t, rc, cc].rearrange("d p -> p d"))
                        rofp_t = gp.tile([128, 1], f32, tag="rofp", name="rofp")
                        nc.sync.dma_start(rofp_t[:], d_rowofp[t, rc, cc].rearrange("(p o) -> p o", o=1))
                        yshm_t = gp.tile([128, TCOL], f32, tag="yshm", name="yshm")
                        nc.sync.dma_start(yshm_t[:], d_yshm[t, rc, cc])
                        xjm_t = gp.tile([128, TCOL], f32, tag="xjm", name="xjm")
                        nc.sync.dma_start(xjm_t[:], d_xjm[t, rc, cc])

                        vars_ = [w0]
                        for dlt in range(1, NY + 1):
                            v_ = gp.tile([128, WCOLS], bf16, tag=f"var{dlt}", name=f"var{dlt}")
                            nc.sync.dma_start(v_[0:128 - dlt, :], w0[dlt:128, :])
                            nc.sync.dma_start(v_[128 - dlt:128, :], w1[0:dlt, :])
                            vars_.append(v_)

                        # ycode = y0f - yshm ; ycode1 = ycode + 1
                        yc32 = wk.tile([128, TCOL], f32, tag="ycode", name="yc32")
                        nc.vector.scalar_tensor_tensor(out=yc32[:], in0=y0f[:], scalar=rofp_t[:, 0:1], in1=yshm_t[:], op0=AL.subtract, op1=AL.subtract)
                        ycode = wk.tile([128, TCOL], bf16, tag="ycb", name="ycode")
                        nc.vector.tensor_copy(out=ycode[:], in_=yc32[:])
                        ycode1 = wk.tile([128, TCOL], bf16, tag="ycb1", name="ycode1")
                        nc.vector.tensor_scalar(out=ycode1[:], in0=ycode[:], scalar1=1.0, scalar2=None, op0=AL.add)
                        # px = wx - xjm in [0,2): u = clamp(px,0,1), v = clamp(px-1,0,1)
                        pxr = wk.tile([128, TCOL], f32, tag="pxr", name="pxr")
                        nc.vector.tensor_tensor(out=pxr[:], in0=x0f[:], in1=xjm_t[:], op=AL.subtract)
                        nc.vector.tensor_tensor(out=pxr[:], in0=pxr[:], in1=fx[:], op=AL.add)
                        uu = wk.tile([128, TCOL], bf16, tag="uu", name="uu")
                        nc.vector.tensor_scalar(out=uu[:], in0=pxr[:], scalar1=0.0, scalar2=1.0, op0=AL.max, op1=AL.min)
                        vv = wk.tile([128, TCOL], bf16, tag="vv", name="vv")
                        nc.vector.tensor_scalar(out=vv[:], in0=pxr[:], scalar1=1.0, scalar2=None, op0=AL.subtract)
                        nc.vector.tensor_scalar(out=vv[:], in0=vv[:], scalar1=0.0, scalar2=1.0, op0=AL.max, op1=AL.min)
                        # c0 = 1-u, c1 = u-v, c2 = v
                        c0t = wk.tile([128, TCOL], bf16, tag="c0t", name="c0t")
                        nc.vector.tensor_scalar(out=c0t[:], in0=uu[:], scalar1=-1.0, scalar2=1.0, op0=AL.mult, op1=AL.add)
                        c1t = wk.tile([128, TCOL], bf16, tag="c1t", name="c1t")
                        nc.vector.tensor_tensor(out=c1t[:], in0=uu[:], in1=vv[:], op=AL.subtract)

                        acc0 = wk.tile([128, TCOL], bf16, tag="acc0", name="acc0")
                        acc1 = wk.tile([128, TCOL], bf16, tag="acc1", name="acc1")
                        xv = wk.tile([128, TCOL], bf16, tag="xv", name="xv")
                        msk = wk.tile([128, TCOL], bf16, tag="mskb", name="msk")
                        tq = wk.tile([128, TCOL], bf16, tag="tqb", name="tq")
                        msk1 = wk.tile([128, TCOL], bf16, tag="mskb1", name="msk1")
                        tq2 = wk.tile([128, TCOL], bf16, tag="tq2b", name="tq2")
                        xvg = wk.tile([128, TCOL], bf16, tag="xvgb", name="xvg")
                        tqg = wk.tile([128, TCOL], bf16, tag="tqgb", name="tqg")
                        for dlt in range(NY + 1):
                            W_ = vars_[dlt]
                            # value chain: even deltas on DVE, odd on GpSimd
                            if True:
                                E, xvv, tqq = (nc.vector, xv, tq) if dlt % 2 == 0 else (nc.vector, xvg, tqg)
                            else:
                                E, xvv, tqq = nc.gpsimd, xvg, tqg
                            E.tensor_tensor(out=xvv[:], in0=c0t[:], in1=W_[:, JW:JW + TCOL], op=AL.mult)
                            E.tensor_tensor(out=tqq[:], in0=c1t[:], in1=W_[:, JW + 1:JW + 1 + TCOL], op=AL.mult)
                            E.tensor_tensor(out=xvv[:], in0=xvv[:], in1=tqq[:], op=AL.add)
                            E.tensor_tensor(out=tqq[:], in0=vv[:], in1=W_[:, JW + 2:JW + 2 + TCOL], op=AL.mult)
                            E.tensor_tensor(out=xvv[:], in0=xvv[:], in1=tqq[:], op=AL.add)
                            A = nc.gpsimd if dlt != 3 else nc.vector
                            if dlt < NY:
                                if dlt == 0:
                                    nc.vector.scalar_tensor_tensor(out=acc0[:], in0=ycode[:], scalar=ytab_t[:, 0:1], in1=xvv[:], op0=AL.is_equal, op1=AL.mult)
                                else:
                                    nc.vector.scalar_tensor_tensor(out=msk[:], in0=ycode[:], scalar=ytab_t[:, dlt:dlt + 1], in1=xvv[:], op0=AL.is_equal, op1=AL.mult)
                                    A.tensor_tensor(out=acc0[:], in0=acc0[:], in1=msk[:], op=AL.add)
                            if dlt == 0:
                                nc.vector.scalar_tensor_tensor(out=acc1[:], in0=ycode1[:], scalar=ytab_t[:, 0:1], in1=xvv[:], op0=AL.is_equal, op1=AL.mult)
                            else:
                                nc.vector.scalar_tensor_tensor(out=msk1[:], in0=ycode1[:], scalar=ytab_t[:, dlt:dlt + 1], in1=xvv[:], op0=AL.is_equal, op1=AL.mult)
                                A.tensor_tensor(out=acc1[:], in0=acc1[:], in1=msk1[:], op=AL.add)

                        # y lerp: A = acc0 + fy*(acc1-acc0)
                        fyb = wk.tile([128, TCOL], bf16, tag="fyb", name="fyb")
                        nc.vector.tensor_copy(out=fyb[:], in_=fy[:])
                        Ab = wk.tile([128, TCOL], bf16, tag="Ab", name="Ab")
                        nc.vector.tensor_tensor(out=Ab[:], in0=acc1[:], in1=acc0[:], op=AL.subtract)
                        nc.vector.tensor_tensor(out=Ab[:], in0=Ab[:], in1=fyb[:], op=AL.mult)
                        nc.vector.tensor_tensor(out=Ab[:], in0=Ab[:], in1=acc0[:], op=AL.add)

                        # far-oob -> 0
                        Af = wk.tile([128, TCOL], f32, tag="Af", name="Af")
                        nc.vector.tensor_copy(out=Af[:], in_=Ab[:])
                        msk2 = wk.tile([128, TCOL], f32, tag="pxr", name="msk2")
                        nc.vector.tensor_scalar(out=msk2[:], in0=oob[:], scalar1=0.0, scalar2=None, op0=AL.is_le)
                        nc.vector.tensor_tensor(out=Af[:], in0=Af[:], in1=msk2[:], op=AL.mult)
                        # row-valid zeroing (global image edges)
                        nc.vector.tensor_scalar(out=Af[:], in0=Af[:], scalar1=rowv_t[:, rc:rc + 1], scalar2=0.0, op0=AL.mult, op1=AL.add)
                        # col-edge zeroing (global edges only)
                        if cc == 0:
                            nc.vector.memset(Af[:, 0:6], 0.0)
                        if cc == 1:
                            nc.vector.memset(Af[:, 966:TCOL], 0.0)

                        # ---------- zncc ----------
                        valid = wk.tile([128, TCOL], f32, tag="dsh", name="valid")
                        nc.vector.tensor_scalar(out=valid[:], in0=Af[:], scalar1=0.0, scalar2=None, op0=AL.not_equal)
                        boxa = wk.tile([128, TCOL], f32, tag="boxr", name="boxa")
                        _box7(nc, tc, wk, boxa, Af, f32, AL, band_t, psp, ctx)
                        tcen = wk.tile([128, TCOL], f32, tag="du2", name="tcen")
                        nc.vector.tensor_tensor(out=tcen[:], in0=Af[:], in1=boxa[:], op=AL.subtract)
                        prod = wk.tile([128, TCOL], f32, tag="prod", name="prod")
                        nc.vector.tensor_tensor(out=prod[:], in0=rcen[:], in1=tcen[:], op=AL.mult)
                        cov = wk.tile([128, TCOL], f32, tag="dv2", name="cov")
                        _box7(nc, tc, wk, cov, prod, f32, AL, band_t, psp, ctx)
                        nc.vector.tensor_tensor(out=prod[:], in0=tcen[:], in1=tcen[:], op=AL.mult)
                        boxtt = wk.tile([128, TCOL], f32, tag="nz", name="boxtt")
                        _box7(nc, tc, wk, boxtt, prod, f32, AL, band_t, psp, ctx)
                        nc.vector.tensor_tensor(out=boxtt[:], in0=boxtt[:], in1=boxrr[:], op=AL.mult)
                        nc.vector.tensor_scalar(out=boxtt[:], in0=boxtt[:], scalar1=EPS, scalar2=None, op0=AL.add)
                        nc.scalar.activation(boxtt[:], boxtt[:], mybir.ActivationFunctionType.Sqrt)
                        nc.vector.reciprocal(boxtt[:], boxtt[:])
                        nc.vector.tensor_tensor(out=cov[:], in0=cov[:], in1=boxtt[:], op=AL.mult)
                        nc.vector.tensor_tensor(out=cov[:], in0=cov[:], in1=valid[:], op=AL.mult)
                        # reduce over valid cols [6, 966)
                        red = wk.tile([128, 1], f32, tag="red", name="red")
                        nc.vector.tensor_reduce(out=red[:], in_=cov[:, 6:966], axis=mybir.AxisListType.X, op=AL.add)
                        nc.vector.tensor_scalar(out=red[:], in0=red[:], scalar1=zmm_t[:, rc:rc + 1], scalar2=0.0, op0=AL.mult, op1=AL.add)
                        nc.vector.tensor_tensor(out=acc[:, t:t + 1], in0=acc[:, t:t + 1], in1=red[:], op=AL.add)
                        nc.vector.tensor_reduce(out=red[:], in_=valid[:, 6:966], axis=mybir.AxisListType.X, op=AL.add)
                        nc.vector.tensor_scalar(out=red[:], in0=red[:], scalar1=zmm_t[:, rc:rc + 1], scalar2=0.0, op0=AL.mult, op1=AL.add)
                        nc.vector.tensor_tensor(out=acc[:, 2 + t:3 + t], in0=acc[:, 2 + t:3 + t], in1=red[:], op=AL.add)

            nc.sync.dma_start(d_acc[:], acc[:])
    nc.finalize()
    return nc


def _box7(nc, tc, wk, out, src, dt_, AL, band_t=None, psp=None, ctx=None):
    """7x7 zero-padded box SUM/49 of src -> out (valid interior only).

    Horizontal: DVE shift-add tree (dtype of src).  Vertical: PE band-matmul;
    PSUM accumulates in f32, the 1/49 scale casts to out's dtype on ACT."""
    import concourse.mybir as mybir
    F = src.shape[1]
    a = wk.tile([128, F], dt_, tag="bx_a", name="bx_a")
    b = wk.tile([128, F], dt_, tag="bx_b", name="bx_b")
    nc.vector.tensor_tensor(out=a[:, 0:F - 1], in0=src[:, 0:F - 1], in1=src[:, 1:F], op=AL.add)
    nc.vector.tensor_tensor(out=b[:, 0:F - 3], in0=a[:, 0:F - 3], in1=a[:, 2:F - 1], op=AL.add)
    nc.vector.tensor_tensor(out=b[:, 0:F - 5], in0=b[:, 0:F - 5], in1=a[:, 4:F - 1], op=AL.add)
    nc.vector.tensor_tensor(out=b[:, 0:F - 6], in0=b[:, 0:F - 6], in1=src[:, 6:F], op=AL.add)
    h = wk.tile([128, F], dt_, tag="bx_h", name="bx_h")
    nc.gpsimd.memset(h[:, 0:3], 0.0)
    nc.gpsimd.memset(h[:, F - 3:F], 0.0)
    nc.scalar.copy(h[:, 3:F - 3], b[:, 0:F - 6])
    half = (F + 1) // 2
    for s0 in range(0, F, half):
        s1 = min(s0 + half, F)
        pt = psp.tile([128, half], mybir.dt.float32, tag="bx_ps", name="bx_ps")
        nc.tensor.matmul(pt[:, 0:s1 - s0], lhsT=band_t[:], rhs=h[:, s0:s1], start=True, stop=True)
        nc.scalar.mul(out[:, s0:s1], pt[:, 0:s1 - s0], 1.0 / 49.0)


def _host_prep(focal, axis_angles, centers, ref_image, ref_depthmap, target_images):
    """Build per-core input maps."""
    f = float(focal)
    cx, cy = W_IMG / 2.0, H_IMG / 2.0
    K = np.array([[f, 0, cx - 0.5], [0, f, cy - 0.5], [0, 0, 1.0]])
    K_inv = np.linalg.inv(K)
    R1 = _rodrigues(np.asarray(axis_angles[0], np.float64))
    C1 = np.asarray(centers[0], np.float64)
    Ms, bs = [], []
    for t in (1, 2):
        Rt = _rodrigues(np.asarray(axis_angles[t], np.float64))
        A = K @ Rt
        Ms.append(A @ R1.T @ K_inv)
        bs.append(A @ (C1 - np.asarray(centers[t], np.float64)))

    # padded targets
    Tpad = np.zeros((2, H_IMG + 2 * PY, W_IMG + 2 * PX), np.float32)
    Tpad[:, PY:PY + H_IMG, PX:PX + W_IMG] = target_images

    dmin = max(float(np.min(ref_depthmap)), 1e-3)
    smin_b = -1.05 / dmin

    def smooth(M, b, xg, yg, sv=0.0):
        w0 = M[0, 0] * xg + M[0, 1] * yg + M[0, 2] + b[0] * sv
        w1 = M[1, 0] * xg + M[1, 1] * yg + M[1, 2] + b[1] * sv
        w2 = M[2, 0] * xg + M[2, 1] * yg + M[2, 2] + b[2] * sv
        return w0 / (w2 + EPS), w1 / (w2 + EPS)

    in_maps = []
    NBLK = TCOL // BLK + 2
    for k in range(NCORES):
        r_lo = k * SHARD
        depth_band = np.zeros((240, 1936), np.float32)
        for i in range(240):
            r = r_lo - 7 + i
            if 0 <= r < H_IMG:
                depth_band[i, 8:8 + W_IMG] = ref_depthmap[r]
        ref_band = np.zeros((238, 1932), np.float32)
        for i in range(238):
            r = r_lo - 6 + i
            if 0 <= r < H_IMG:
                ref_band[i, 6:6 + W_IMG] = ref_image[r]

        win = np.zeros((2, 2, 2, WROWS, WCOLS), np.float32)
        ytab = np.zeros((2, 2, 2, NY + 1, 128), np.float32)
        rowofp = np.zeros((2, 2, 2, 128), np.float32)
        yshm = np.zeros((2, 2, 2, 128, TCOL), np.float32)
        xjm = np.zeros((2, 2, 2, 128, TCOL), np.float32)
        jabs = np.zeros((2, 128, TCOL), np.float32)
        cxm = np.zeros((2, 128, TCOL), np.float32)
        cym = np.zeros((2, 128), np.float32)
        ryv = np.zeros((2, 3, 2, 128), np.float32)
        rowv = np.zeros((2, 128), np.float32)
        zmm = np.zeros((2, 128), np.float32)
        sc = np.zeros((32, 128), np.float32)
        sc[0, :] = (f / 2.0) ** 2
        for ti in range(2):
            for i in range(3):
                sc[1 + ti * 3 + i, :] = bs[ti][i]
                sc[7 + ti * 3 + i, :] = Ms[ti][i, 0]

        for cc in range(2):
            X = C0S[cc] - 6 + np.arange(TCOL, dtype=np.float64)
            jabs[cc, :, :] = X[None, :].astype(np.float32)
            cxm[cc, :, :] = (cx - X)[None, :].astype(np.float32)
        for rc in range(2):
            Yrows = r_lo + R0S[rc] - 6 + np.arange(128, dtype=np.float64)
            cym[rc, :] = (cy - Yrows).astype(np.float32)
            rowv[rc, :] = ((Yrows >= 0) & (Yrows < H_IMG)).astype(np.float32)
            p = np.arange(128)
            lo, hi = (6, 122) if rc == 0 else (12, 76)
            shard_ok = (Yrows >= r_lo) & (Yrows < r_lo + SHARD)
            zmm[rc, :] = ((p >= lo) & (p < hi) & shard_ok).astype(np.float32)
            for ti in range(2):
                for i in range(3):
                    ryv[ti, i, rc, :] = (Ms[ti][i, 1] * Yrows + Ms[ti][i, 2]).astype(np.float32)

        for ti in range(2):
            M, b = Ms[ti], bs[ti]
            # typical (bulk) jitter offset from a mid-range s
            _, jy_tail = [u - v for u, v in zip(
                smooth(M, b, cx, cy, smin_b), smooth(M, b, cx, cy, 0.0))]
            for rc in range(2):
                Yrows = r_lo + R0S[rc] - 6 + np.arange(WROWS, dtype=np.float64)
                for cc in range(2):
                    X = C0S[cc] - 6 + np.arange(TCOL, dtype=np.float64)
                    xc_mid = X[TCOL // 2]
                    yc_mid = Yrows[64]
                    # column shear (y): per-BLK block
                    wy_c = smooth(M, b, X, yc_mid)[1]
                    wy_mid = smooth(M, b, xc_mid, yc_mid)[1]
                    nblk = (WCOLS + BLK - 1) // BLK
                    yshb = np.zeros(nblk)
                    for bi in range(nblk):
                        c0b = min(bi * BLK + BLK // 2, TCOL - 1)
                        yshb[bi] = np.round(wy_c[c0b] - wy_mid)
                    yshm_row = yshb[(np.arange(TCOL) // BLK)]
                    yshm[ti, rc, cc, :, :] = yshm_row[None, :].astype(np.float32)
                    # row map: rowOf(i) = round(wy_smooth(Y(i), xc_mid)) + bias
                    wy_i = smooth(M, b, xc_mid, Yrows)[1]
                    bias = -1.0 if jy_tail < 0 else 0.0
                    rowOf = np.round(wy_i + bias - 1.0)
                    rowofp[ti, rc, cc, :] = rowOf[0:128].astype(np.float32)
                    for d in range(NY + 1):
                        idx = np.arange(128) + d
                        ytab[ti, rc, cc, d, :] = (rowOf[idx] - rowOf[0:128]).astype(np.float32)
                    # x shear baked per block at row center: colOf(c)
                    wx_c = smooth(M, b, X, yc_mid)[0]
                    xsh = np.round(wx_c - X)  # smooth disp per col
                    xshb = np.zeros(nblk)
                    for bi in range(nblk):
                        c0b = min(bi * BLK + BLK // 2, TCOL - 1)
                        xshb[bi] = xsh[c0b]
                    colsh = xshb[(np.arange(WCOLS) // BLK)]
                    colOf = (np.arange(WCOLS) - JW + X[0] + colsh).astype(np.int64)
                    xjm[ti, rc, cc, :, :] = colOf[JW:JW + TCOL][None, :].astype(np.float32) - 1.0
                    # window content
                    rows_i = rowOf.astype(np.int64)
                    ysh_cols = yshm_row.astype(np.int64)
                    # win[i, c] = Tpad[rowOf(i) + ysh(c) + PY, colOf(c) + PX]
                    ri = np.clip(rows_i[:, None] + np.concatenate([ysh_cols, np.full(WCOLS - TCOL, ysh_cols[-1])])[None, :WCOLS].astype(np.int64) + PY, 0, H_IMG + 2 * PY - 1)
                    ci = np.clip(colOf[None, :] + PX, 0, W_IMG + 2 * PX - 1)
                    win[ti, rc, cc] = Tpad[ti][ri, np.broadcast_to(ci, ri.shape)]

        band7 = np.zeros((128, 128), np.float32)
        for i_ in range(128):
            for j_ in range(max(0, i_ - 3), min(128, i_ + 4)):
                band7[i_, j_] = 1.0
        in_maps.append({
            "depth_band": depth_band, "ref_band": ref_band, "win": win, "band7": band7, "rowofp": rowofp,
            "ytab": ytab, "yshm": yshm, "xjm": xjm, "jabs": jabs,
            "cxm": cxm, "cym": cym, "sc": sc, "ryv": ryv,
            "rowv": rowv, "zmm": zmm,
        })
    return in_maps


def kernel(focal, axis_angles, centers, ref_image, ref_depthmap, target_images):
    from concourse.bass_utils import run_bass_kernel_spmd

    focal = np.asarray(focal, np.float32)
    axis_angles = np.asarray(axis_angles, np.float32)
    centers = np.asarray(centers, np.float32)
    ref_image = np.asarray(ref_image, np.float32)
    ref_depthmap = np.asarray(ref_depthmap, np.float32)
    target_images = np.asarray(target_images, np.float32)

    if "nc" not in _cache:
        _cache["nc"] = _build_program()
    nc = _cache["nc"]

    in_maps = _host_prep(focal, axis_angles, centers, ref_image,
                         ref_depthmap, target_images)
    res = run_bass_kernel_spmd(nc, in_maps, list(range(NCORES)))
    _cache["last_results"] = res

    total_score = np.float32(0.0)
    total_pixels = np.float32(0.0)
    for k in range(NCORES):
        a = res.results[k]["acc"]
        total_score += a[:, 0].sum(dtype=np.float32) + a[:, 1].sum(dtype=np.float32)
        total_pixels += a[:, 2].sum(dtype=np.float32) + a[:, 3].sum(dtype=np.float32)
    mean_zncc = total_score / max(total_pixels, np.float32(1.0))
    loss = np.float32(0.5) * (np.float32(1.0) - mean_zncc) if total_pixels > 0 else np.float32(0.0)
    return np.float32(loss)



# revision 14
# speedup vs baseline: 1.3741x; 1.3741x over previous
"""Trainium2 Bass kernel for the plane-sweep ZNCC photometric loss module.

Contract: kernel(**inputs) takes the FULL unsharded inputs and returns the
full output (a scalar float32 loss).  Internally the (H, W) pixel grid is
sharded across 8 NeuronCores along H (180 rows per core).  Each core
computes surface normals, the per-pixel homography warp, bilinear sampling
of the target images (via a window-gather scheme), windowed 7x7 ZNCC, and
partial (score, count) sums; the host adds the 16 partial scalars and
finishes the loss.

v2: 7x7 box filters run entirely on the tensor engine (7 shifted matmuls
accumulating in PSUM against a banded 0/1 matrix), the ZNCC chain runs in
bf16, reciprocals use the fast DVE approximation, and elementwise work is
spread across the Vector/Scalar/GpSimd engines.
"""

import numpy as np
import ml_dtypes

H_IMG, W_IMG = 1440, 1920
NCORES = 8
SHARD = H_IMG // NCORES          # 180 rows per core

# chunk geometry (per core): 2 row-chunks x 2 col-chunks
R0S = [0, 110]                   # tile-row offsets inside the shard band
C0S = [0, 960]                   # absolute col origins
TCOL = 972                       # tile cols = 960 + 12 halo
NY = 3                           # y-combo count (window row candidates)
WROWS = 144                      # window rows per chunk (128 + NY + margins)
WCOLS = 980                      # window cols (972 + x-margin)
JW = 3                           # window col of output col q at dx=0
PY, PX = 160, 96                 # zero padding around targets
BLK = 8                          # shear block (cols)
EPS = 1e-8

_cache = {}


def _rodrigues(v):
    th = np.linalg.norm(v) + 1e-12
    k = v / th
    Kx = np.array([[0, -k[2], k[1]], [k[2], 0, -k[0]], [-k[1], k[0], 0]])
    return np.eye(3) + np.sin(th) * Kx + (1 - np.cos(th)) * (Kx @ Kx)


def _build_program():
    """Build the (input-independent) SPMD bass program."""
    import concourse.bacc as bacc
    import concourse.mybir as mybir
    from concourse import tile

    f32 = mybir.dt.float32
    bf16 = mybir.dt.bfloat16
    AL = mybir.AluOpType
    ACT = mybir.ActivationFunctionType

    nc = bacc.Bacc(None, target_bir_lowering=False)

    # ---- DRAM inputs (per core) ----
    d_depth = nc.dram_tensor("depth_band", [240, 1936], f32, kind="ExternalInput")
    d_ref = nc.dram_tensor("ref_band", [238, 1932], bf16, kind="ExternalInput")
    d_win = nc.dram_tensor("win", [2, 2, 2, WROWS, WCOLS], bf16, kind="ExternalInput")
    d_ytab = nc.dram_tensor("ytab", [2, 2, 2, NY + 1, 128], f32, kind="ExternalInput")
    d_yshm = nc.dram_tensor("yshm", [2, 2, 2, 128, TCOL], f32, kind="ExternalInput")
    d_xjm = nc.dram_tensor("xjm", [2, 2, 2, 128, TCOL], f32, kind="ExternalInput")
    d_jabs = nc.dram_tensor("jabs", [2, 128, TCOL], f32, kind="ExternalInput")
    d_cxm = nc.dram_tensor("cxm", [2, 128, TCOL], f32, kind="ExternalInput")
    d_cym = nc.dram_tensor("cym", [2, 128], f32, kind="ExternalInput")
    d_sc = nc.dram_tensor("sc", [32, 128], f32, kind="ExternalInput")
    # sc rows: 0: fsq=(f/2)^2, 1..6: b[t][i] (t-major), 7..12: M[t][i,0],
    # 13: EPS, 14: 0.0, 15: 1e-12, 16: 1/hx,
    # 17: -cx/hx, 18: 1/hy, 19: -cy/hy, 20: 0.0, 21: -1.0, 22: +1.0
    d_ryv = nc.dram_tensor("ryv", [2, 3, 2, 128], f32, kind="ExternalInput")
    d_rowv = nc.dram_tensor("rowv", [2, 128], f32, kind="ExternalInput")
    d_zmm = nc.dram_tensor("zmm", [2, 128], f32, kind="ExternalInput")

    d_band = nc.dram_tensor("band7", [128, 128], bf16, kind="ExternalInput")
    d_rowofp = nc.dram_tensor("rowofp", [2, 2, 2, 128], f32, kind="ExternalInput")

    d_acc = nc.dram_tensor("acc", [128, 4], f32, kind="ExternalOutput")

    with tile.TileContext(nc) as tc:
        with tc.tile_pool(name="io", bufs=1) as iop, \
             tc.tile_pool(name="wk", bufs=1) as wk, \
             tc.tile_pool(name="gat", bufs=2) as gp, \
             tc.tile_pool(name="ps", bufs=2, space="PSUM") as psp:

            acc = iop.tile([128, 4], f32)
            nc.vector.memset(acc[:], 0.0)

            jabs_t = [iop.tile([128, TCOL], f32, tag=f"jabs{c}", name=f"jabs{c}") for c in range(2)]
            cxm_t = [iop.tile([128, TCOL], f32, tag=f"cxm{c}", name=f"cxm{c}") for c in range(2)]
            for c in range(2):
                nc.sync.dma_start(jabs_t[c][:], d_jabs[c])
                nc.sync.dma_start(cxm_t[c][:], d_cxm[c])
            cym_t = iop.tile([128, 2], f32)
            nc.sync.dma_start(cym_t[:], d_cym.rearrange("r p -> p r"))
            sc_t = iop.tile([128, 32], f32)
            nc.sync.dma_start(sc_t[:], d_sc.rearrange("r p -> p r"))
            ryv_t = iop.tile([128, 12], f32)
            nc.sync.dma_start(ryv_t[:], d_ryv.rearrange("t i r p -> p (t i r)"))
            rowv_t = iop.tile([128, 2], f32)
            nc.sync.dma_start(rowv_t[:], d_rowv.rearrange("r p -> p r"))
            zmm_t = iop.tile([128, 2], f32)
            nc.sync.dma_start(zmm_t[:], d_zmm.rearrange("r p -> p r"))
            band_t = iop.tile([128, 128], bf16)
            nc.sync.dma_start(band_t[:], d_band[:])

            def box7(out, src, tagp):
                """7x7 box mean of src -> out, valid cols [3, 969) only.
                Pure PE: 7 shifted matmuls accumulate in PSUM; ScalarE
                scales 1/49 out of PSUM."""
                for lo, hi in ((3, 486), (486, 969)):
                    pt = psp.tile([128, 483], f32, tag="bx_ps", name="bx_ps")
                    for dx in range(7):
                        nc.tensor.matmul(pt[:, 0:hi - lo], lhsT=band_t[:],
                                         rhs=src[:, lo - 3 + dx:hi - 3 + dx],
                                         start=(dx == 0), stop=(dx == 6))
                    nc.scalar.mul(out[:, lo:hi], pt[:, 0:hi - lo], 1.0 / 49.0)

            for rc in range(2):
                for cc in range(2):
                    R0 = R0S[rc]
                    # ---------- load depth/ref ----------
                    dptA = wk.tile([128, 974], f32, tag="dptA", name="dptA", bufs=2)
                    nc.sync.dma_start(dptA[:], d_depth[R0:R0 + 128, C0S[cc]:C0S[cc] + 974])
                    dptB = wk.tile([2, 974], f32, tag="dptB", name="dptB")
                    nc.sync.dma_start(dptB[:], d_depth[R0 + 128:R0 + 130, C0S[cc]:C0S[cc] + 974])
                    dsh = wk.tile([128, 974], f32, tag="dsh", name="dsh")
                    nc.sync.dma_start(dsh[0:126, :], dptA[2:128, :])
                    nc.sync.dma_start(dsh[126:128, :], dptB[0:2, :])
                    rt = wk.tile([128, TCOL], bf16, tag="rt", name="rt", bufs=2)
                    nc.sync.dma_start(rt[:], d_ref[R0:R0 + 128, C0S[cc]:C0S[cc] + TCOL])

                    # ---------- s field (shared by both targets) ----------
                    # du2 = d(X+1)-d(X-1), dv2 = d(Y+1)-d(Y-1)   [128, TCOL]
                    du2 = wk.tile([128, TCOL], f32, tag="du2", name="du2")
                    nc.vector.tensor_tensor(out=du2[:], in0=dptA[:, 2:974], in1=dptA[:, 0:TCOL], op=AL.subtract)
                    dv2 = wk.tile([128, TCOL], f32, tag="dv2", name="dv2")
                    nc.vector.tensor_tensor(out=dv2[:], in0=dsh[:, 1:973], in1=dptA[:, 1:973], op=AL.subtract)
                    dep = dptA[:, 1:973]  # depth at the pixel
                    # nz = 0.5*(cxm*du2 + cym*dv2) - dep
                    t1 = wk.tile([128, TCOL], f32, tag="t1", name="t1")
                    nc.vector.tensor_tensor(out=t1[:], in0=cxm_t[cc][:], in1=du2[:], op=AL.mult)
                    nc.vector.scalar_tensor_tensor(out=t1[:], in0=dv2[:], scalar=cym_t[:, rc:rc + 1], in1=t1[:], op0=AL.mult, op1=AL.add)
                    nz = wk.tile([128, TCOL], f32, tag="nz", name="nz")
                    nc.vector.scalar_tensor_tensor(out=nz[:], in0=t1[:], scalar=0.5, in1=dep, op0=AL.mult, op1=AL.subtract)
                    # n2 = nz^2 + fsq*(du2^2 + dv2^2)   (squares on ScalarE)
                    d2u = wk.tile([128, TCOL], f32, tag="d2u", name="d2u")
                    nc.scalar.activation(d2u[:], du2[:], ACT.Square, bias=sc_t[:, 14:15], scale=1.0)
                    d2v = wk.tile([128, TCOL], f32, tag="d2v", name="d2v")
                    nc.scalar.activation(d2v[:], dv2[:], ACT.Square, bias=sc_t[:, 14:15], scale=1.0)
                    nz2 = wk.tile([128, TCOL], f32, tag="nz2", name="nz2")
                    nc.scalar.activation(nz2[:], nz[:], ACT.Square, bias=sc_t[:, 14:15], scale=1.0)
                    n2 = wk.tile([128, TCOL], f32, tag="n2", name="n2")
                    nc.gpsimd.tensor_tensor(out=n2[:], in0=d2u[:], in1=d2v[:], op=AL.add)
                    nc.vector.scalar_tensor_tensor(out=n2[:], in0=n2[:], scalar=sc_t[:, 0:1], in1=nz2[:], op0=AL.mult, op1=AL.add)
                    # nrm = sqrt(n2 + tiny);  den = nrm * dep;  rec = 1/den
                    nrm = wk.tile([128, TCOL], f32, tag="nrm", name="nrm")
                    nc.scalar.activation(nrm[:], n2[:], ACT.Sqrt, bias=sc_t[:, 15:16], scale=1.0)
                    den = wk.tile([128, TCOL], f32, tag="den", name="den")
                    nc.vector.scalar_tensor_tensor(out=den[:], in0=dep, scalar=EPS, in1=nrm[:], op0=AL.add, op1=AL.mult)
                    nc.vector.reciprocal_approx_fast(out=den[:], in_=den[:])
                    # sfield = (0.25*(du2+dv2) - dep) * rec
                    sfield = wk.tile([128, TCOL], f32, tag="sfield", name="sfield")
                    nc.vector.tensor_tensor(out=sfield[:], in0=du2[:], in1=dv2[:], op=AL.add)
                    nc.vector.scalar_tensor_tensor(out=sfield[:], in0=sfield[:], scalar=0.25, in1=dep, op0=AL.mult, op1=AL.subtract)
                    nc.vector.tensor_tensor(out=sfield[:], in0=sfield[:], in1=den[:], op=AL.mult)

                    # shared ref box terms for this chunk (bf16, PE box)
                    boxr = wk.tile([128, TCOL], bf16, tag="boxr", name="boxr")
                    box7(boxr, rt, "br")
                    rcen = wk.tile([128, TCOL], bf16, tag="rcen", name="rcen")
                    nc.vector.tensor_tensor(out=rcen[:, 3:969], in0=rt[:, 3:969], in1=boxr[:, 3:969], op=AL.subtract)
                    rc2 = wk.tile([128, TCOL], bf16, tag="rc2", name="rc2")
                    nc.scalar.activation(rc2[:, 3:969], rcen[:, 3:969], ACT.Square, bias=sc_t[:, 14:15], scale=1.0)
                    nc.gpsimd.memset(rc2[:, 0:3], 0.0)
                    nc.gpsimd.memset(rc2[:, 969:TCOL], 0.0)
                    boxrr = wk.tile([128, TCOL], bf16, tag="boxrr", name="boxrr")
                    box7(boxrr, rc2, "brr")

                    for t in range(2):
                        # ---------- warp ----------
                        ww = []
                        for i in range(3):
                            w_ = wk.tile([128, TCOL], f32, tag=f"w{i}", name=f"w{i}")
                            # w = M[i,0]*jabs + ryv   (ScalarE, per-partition scale+bias)
                            nc.scalar.activation(
                                w_[:], jabs_t[cc][:], ACT.Identity,
                                bias=ryv_t[:, t * 6 + i * 2 + rc:t * 6 + i * 2 + rc + 1],
                                scale=sc_t[:, 7 + t * 3 + i:8 + t * 3 + i])
                            # += b[i]*s
                            nc.vector.scalar_tensor_tensor(
                                out=w_[:], in0=sfield[:],
                                scalar=sc_t[:, 1 + t * 3 + i:2 + t * 3 + i],
                                in1=w_[:], op0=AL.mult, op1=AL.add)
                            ww.append(w_)
                        rec = wk.tile([128, TCOL], f32, tag="rec", name="rec")
                        nc.vector.tensor_scalar(out=rec[:], in0=ww[2][:], scalar1=EPS, scalar2=1e-6, op0=AL.add, op1=AL.max)
                        nc.vector.reciprocal_approx_fast(out=rec[:], in_=rec[:])
                        wx = wk.tile([128, TCOL], f32, tag="wx", name="wx")
                        nc.vector.tensor_tensor(out=wx[:], in0=ww[0][:], in1=rec[:], op=AL.mult)
                        wy = wk.tile([128, TCOL], f32, tag="wy", name="wy")
                        nc.gpsimd.tensor_tensor(out=wy[:], in0=ww[1][:], in1=rec[:], op=AL.mult)

                        # oob: big = max(|wx-cx|/hx, |wy-cy|/hy) ; oob iff big > 1
                        axx = wk.tile([128, TCOL], f32, tag="w0", name="axx")
                        nc.scalar.activation(axx[:], wx[:], ACT.Abs,
                                             bias=sc_t[:, 17:18], scale=sc_t[:, 16:17])
                        ayy = wk.tile([128, TCOL], f32, tag="w1", name="ayy")
                        nc.scalar.activation(ayy[:], wy[:], ACT.Abs,
                                             bias=sc_t[:, 19:20], scale=sc_t[:, 18:19])
                        big = wk.tile([128, TCOL], f32, tag="w2", name="big")
                        nc.vector.tensor_tensor(out=big[:], in0=axx[:], in1=ayy[:], op=AL.max)

                        # ---------- y code / fraction ----------
                        # floor(wy) via round-nearest cast of (wy - 0.5);
                        # exact-tie cases are measure-zero
                        flo = wk.tile([128, TCOL], f32, tag="flo", name="flo")
                        nc.vector.tensor_scalar(out=flo[:], in0=wy[:], scalar1=0.5, scalar2=None, op0=AL.subtract)
                        iti = wk.tile([128, TCOL], mybir.dt.int32, tag="iti", name="iti")
                        nc.vector.tensor_copy(out=iti[:], in_=flo[:])
                        nc.vector.tensor_copy(out=flo[:], in_=iti[:])
                        fyb = wk.tile([128, TCOL], bf16, tag="fyb", name="fyb")
                        nc.vector.tensor_tensor(out=fyb[:], in0=wy[:], in1=flo[:], op=AL.subtract)
                        # ycode = floor(wy) - rofp - yshm
                        rofp_t = gp.tile([128, 1], f32, tag="rofp", name="rofp")
                        nc.sync.dma_start(rofp_t[:], d_rowofp[t, rc, cc].rearrange("(p o) -> p o", o=1))
                        yshm_t = gp.tile([128, TCOL], f32, tag="yshm", name="yshm")
                        nc.sync.dma_start(yshm_t[:], d_yshm[t, rc, cc])
                        ycode = wk.tile([128, TCOL], bf16, tag="ycb", name="ycode")
                        nc.vector.scalar_tensor_tensor(out=ycode[:], in0=flo[:], scalar=rofp_t[:, 0:1], in1=yshm_t[:], op0=AL.subtract, op1=AL.subtract)
                        ycode1 = wk.tile([128, TCOL], bf16, tag="ycb1", name="ycode1")
                        nc.vector.tensor_scalar(out=ycode1[:], in0=ycode[:], scalar1=1.0, scalar2=None, op0=AL.add)

                        # ---------- x fraction / 3-tap weights ----------
                        xjm_t = gp.tile([128, TCOL], f32, tag="xjm", name="xjm")
                        nc.sync.dma_start(xjm_t[:], d_xjm[t, rc, cc])
                        pxr = wk.tile([128, TCOL], f32, tag="pxr", name="pxr")
                        nc.vector.tensor_tensor(out=pxr[:], in0=wx[:], in1=xjm_t[:], op=AL.subtract)
                        # c0 = relu(1 - relu(px)); vv = relu(px - 1); negc1 = c0 + vv - 1
                        u1 = wk.tile([128, TCOL], f32, tag="u1", name="u1")
                        nc.scalar.activation(u1[:], pxr[:], ACT.Relu, bias=sc_t[:, 20:21], scale=1.0)
                        c0t = wk.tile([128, TCOL], bf16, tag="c0t", name="c0t")
                        nc.scalar.activation(c0t[:], u1[:], ACT.Relu, bias=sc_t[:, 22:23], scale=-1.0)
                        vv = wk.tile([128, TCOL], bf16, tag="vv", name="vv")
                        nc.scalar.activation(vv[:], pxr[:], ACT.Relu, bias=sc_t[:, 21:22], scale=1.0)
                        negc1 = wk.tile([128, TCOL], bf16, tag="negc1", name="negc1")
                        nc.vector.scalar_tensor_tensor(out=negc1[:], in0=c0t[:], scalar=-1.0, in1=vv[:], op0=AL.add, op1=AL.add)

                        # ---------- gather ----------
                        w0 = gp.tile([128, WCOLS], bf16, tag="w0", name="w0", bufs=2)
                        nc.sync.dma_start(w0[:], d_win[t, rc, cc, 0:128, :])
                        w1 = gp.tile([16, WCOLS], bf16, tag="w1", name="w1", bufs=2)
                        nc.sync.dma_start(w1[:], d_win[t, rc, cc, 128:WROWS, :])
                        ytab_t = gp.tile([128, NY + 1], f32, tag="ytab", name="ytab")
                        nc.sync.dma_start(ytab_t[:], d_ytab[t, rc, cc].rearrange("d p -> p d"))

                        vars_ = [w0]
                        for dlt in range(1, NY + 1):
                            v_ = gp.tile([128, WCOLS], bf16, tag=f"var{dlt}", name=f"var{dlt}")
                            nc.sync.dma_start(v_[0:128 - dlt, :], w0[dlt:128, :])
                            nc.sync.dma_start(v_[128 - dlt:128, :], w1[0:dlt, :])
                            vars_.append(v_)

                        acc0 = wk.tile([128, TCOL], bf16, tag="acc0", name="acc0")
                        acc1 = wk.tile([128, TCOL], bf16, tag="acc1", name="acc1")
                        for dlt in range(NY + 1):
                            W_ = vars_[dlt]
                            xv = wk.tile([128, TCOL], bf16, tag="xv", name=f"xv{dlt}", bufs=2)
                            tq = wk.tile([128, TCOL], bf16, tag="tqb", name=f"tq{dlt}", bufs=2)
                            # xv = c0*W0 - negc1*W1 + vv*W2
                            nc.vector.tensor_tensor(out=xv[:], in0=c0t[:], in1=W_[:, JW:JW + TCOL], op=AL.mult)
                            nc.vector.tensor_tensor(out=tq[:], in0=negc1[:], in1=W_[:, JW + 1:JW + 1 + TCOL], op=AL.mult)
                            nc.vector.tensor_tensor(out=xv[:], in0=xv[:], in1=tq[:], op=AL.subtract)
                            nc.vector.tensor_tensor(out=tq[:], in0=vv[:], in1=W_[:, JW + 2:JW + 2 + TCOL], op=AL.mult)
                            nc.vector.tensor_tensor(out=xv[:], in0=xv[:], in1=tq[:], op=AL.add)
                            if dlt < NY:
                                if dlt == 0:
                                    nc.vector.scalar_tensor_tensor(out=acc0[:], in0=ycode[:], scalar=ytab_t[:, 0:1], in1=xv[:], op0=AL.is_equal, op1=AL.mult)
                                else:
                                    msk = wk.tile([128, TCOL], bf16, tag="mskb", name=f"msk{dlt}", bufs=2)
                                    nc.vector.scalar_tensor_tensor(out=msk[:], in0=ycode[:], scalar=ytab_t[:, dlt:dlt + 1], in1=xv[:], op0=AL.is_equal, op1=AL.mult)
                                    nc.gpsimd.tensor_tensor(out=acc0[:], in0=acc0[:], in1=msk[:], op=AL.add)
                            if dlt == 0:
                                nc.vector.scalar_tensor_tensor(out=acc1[:], in0=ycode1[:], scalar=ytab_t[:, 0:1], in1=xv[:], op0=AL.is_equal, op1=AL.mult)
                            else:
                                A = nc.gpsimd if dlt == 2 else nc.vector
                                msk1 = wk.tile([128, TCOL], bf16, tag="mskb1", name=f"msk1_{dlt}", bufs=2)
                                nc.vector.scalar_tensor_tensor(out=msk1[:], in0=ycode1[:], scalar=ytab_t[:, dlt:dlt + 1], in1=xv[:], op0=AL.is_equal, op1=AL.mult)
                                A.tensor_tensor(out=acc1[:], in0=acc1[:], in1=msk1[:], op=AL.add)

                        # y lerp: A = acc0 + fy*(acc1-acc0)
                        Ab = wk.tile([128, TCOL], bf16, tag="Ab", name="Ab")
                        nc.vector.tensor_tensor(out=Ab[:], in0=acc1[:], in1=acc0[:], op=AL.subtract)
                        nc.vector.tensor_tensor(out=Ab[:], in0=Ab[:], in1=fyb[:], op=AL.mult)
                        Abf = wk.tile([128, TCOL], f32, tag="flo", name="Abf")
                        nc.vector.tensor_tensor(out=Abf[:], in0=Ab[:], in1=acc0[:], op=AL.add)

                        # oob -> 0  (big <= 1 keeps), then row-valid scaling
                        Af = wk.tile([128, TCOL], bf16, tag="Af", name="Af")
                        nc.vector.scalar_tensor_tensor(out=Af[:], in0=big[:], scalar=1.0, in1=Abf[:], op0=AL.is_le, op1=AL.mult)
                        nc.scalar.activation(Af[:], Af[:], ACT.Copy, bias=0.0, scale=rowv_t[:, rc:rc + 1])
                        # col-edge zeroing (global edges only)
                        if cc == 0:
                            nc.gpsimd.memset(Af[:, 0:6], 0.0)
                        if cc == 1:
                            nc.gpsimd.memset(Af[:, 966:TCOL], 0.0)

                        # ---------- zncc ----------
                        valid = wk.tile([128, TCOL], f32, tag="dsh", name="valid")
                        red0 = wk.tile([128, 1], f32, tag="red0", name="red0")
                        nc.vector.tensor_scalar(out=valid[:, 6:966], in0=Af[:, 6:966], scalar1=0.0, scalar2=None, op0=AL.not_equal)
                        nc.vector.tensor_reduce(out=red0[:], in_=valid[:, 6:966], axis=mybir.AxisListType.X, op=AL.add)
                        boxa = wk.tile([128, TCOL], bf16, tag="boxa", name="boxa")
                        box7(boxa, Af, "ba")
                        tcen = wk.tile([128, TCOL], bf16, tag="du2", name="tcen")
                        nc.vector.tensor_tensor(out=tcen[:, 3:969], in0=Af[:, 3:969], in1=boxa[:, 3:969], op=AL.subtract)
                        prod = wk.tile([128, TCOL], bf16, tag="dv2", name="prod")
                        nc.vector.tensor_tensor(out=prod[:, 3:969], in0=rcen[:, 3:969], in1=tcen[:, 3:969], op=AL.mult)
                        nc.gpsimd.memset(prod[:, 0:3], 0.0)
                        nc.gpsimd.memset(prod[:, 969:TCOL], 0.0)
                        cov = wk.tile([128, TCOL], f32, tag="t1", name="cov")
                        box7(cov, prod, "cv")
                        tsq = wk.tile([128, TCOL], bf16, tag="nz", name="tsq")
                        nc.scalar.activation(tsq[:, 3:969], tcen[:, 3:969], ACT.Square, bias=sc_t[:, 14:15], scale=1.0)
                        nc.gpsimd.memset(tsq[:, 0:3], 0.0)
                        nc.gpsimd.memset(tsq[:, 969:TCOL], 0.0)
                        boxtt = wk.tile([128, TCOL], bf16, tag="n2", name="boxtt")
                        box7(boxtt, tsq, "bt")
                        # denom = sqrt(boxtt*boxrr + eps); zm = cov/denom * valid
                        den3 = wk.tile([128, TCOL], f32, tag="d2u", name="den3")
                        nc.vector.tensor_tensor(out=den3[:, 6:966], in0=boxtt[:, 6:966], in1=boxrr[:, 6:966], op=AL.mult)
                        nc.scalar.activation(den3[:, 6:966], den3[:, 6:966], ACT.Sqrt, bias=sc_t[:, 13:14], scale=1.0)
                        nc.vector.reciprocal_approx_fast(out=den3[:, 6:966], in_=den3[:, 6:966])
                        zs = wk.tile([128, TCOL], f32, tag="d2v", name="zs")
                        nc.vector.tensor_tensor(out=zs[:, 6:966], in0=cov[:, 6:966], in1=den3[:, 6:966], op=AL.mult)
                        zv = wk.tile([128, TCOL], f32, tag="nz2", name="zv")
                        red1 = wk.tile([128, 1], f32, tag="red1", name="red1")
                        nc.vector.tensor_tensor(out=zv[:, 6:966], in0=zs[:, 6:966], in1=valid[:, 6:966], op=AL.mult)
                        nc.vector.tensor_reduce(out=red1[:], in_=zv[:, 6:966], axis=mybir.AxisListType.X, op=AL.add)
                        # zmm row-mask + accumulate
                        nc.vector.tensor_scalar(out=red1[:], in0=red1[:], scalar1=zmm_t[:, rc:rc + 1], scalar2=0.0, op0=AL.mult, op1=AL.add)
                        nc.vector.tensor_tensor(out=acc[:, t:t + 1], in0=acc[:, t:t + 1], in1=red1[:], op=AL.add)
                        nc.vector.tensor_scalar(out=red0[:], in0=red0[:], scalar1=zmm_t[:, rc:rc + 1], scalar2=0.0, op0=AL.mult, op1=AL.add)
                        nc.vector.tensor_tensor(out=acc[:, 2 + t:3 + t], in0=acc[:, 2 + t:3 + t], in1=red0[:], op=AL.add)

            nc.sync.dma_start(d_acc[:], acc[:])
    nc.finalize()
    return nc


def _host_prep(focal, axis_angles, centers, ref_image, ref_depthmap, target_images):
    """Build per-core input maps."""
    f = float(focal)
    cx, cy = W_IMG / 2.0, H_IMG / 2.0
    K = np.array([[f, 0, cx - 0.5], [0, f, cy - 0.5], [0, 0, 1.0]])
    K_inv = np.linalg.inv(K)
    R1 = _rodrigues(np.asarray(axis_angles[0], np.float64))
    C1 = np.asarray(centers[0], np.float64)
    Ms, bs = [], []
    for t in (1, 2):
        Rt = _rodrigues(np.asarray(axis_angles[t], np.float64))
        A = K @ Rt
        Ms.append(A @ R1.T @ K_inv)
        bs.append(A @ (C1 - np.asarray(centers[t], np.float64)))

    # padded targets
    Tpad = np.zeros((2, H_IMG + 2 * PY, W_IMG + 2 * PX), np.float32)
    Tpad[:, PY:PY + H_IMG, PX:PX + W_IMG] = target_images

    dmin = max(float(np.min(ref_depthmap)), 1e-3)
    smin_b = -1.05 / dmin

    def smooth(M, b, xg, yg, sv=0.0):
        w0 = M[0, 0] * xg + M[0, 1] * yg + M[0, 2] + b[0] * sv
        w1 = M[1, 0] * xg + M[1, 1] * yg + M[1, 2] + b[1] * sv
        w2 = M[2, 0] * xg + M[2, 1] * yg + M[2, 2] + b[2] * sv
        return w0 / (w2 + EPS), w1 / (w2 + EPS)

    in_maps = []
    for k in range(NCORES):
        r_lo = k * SHARD
        depth_band = np.zeros((240, 1936), np.float32)
        for i in range(240):
            r = r_lo - 7 + i
            if 0 <= r < H_IMG:
                depth_band[i, 8:8 + W_IMG] = ref_depthmap[r]
        ref_band = np.zeros((238, 1932), np.float32)
        for i in range(238):
            r = r_lo - 6 + i
            if 0 <= r < H_IMG:
                ref_band[i, 6:6 + W_IMG] = ref_image[r]

        win = np.zeros((2, 2, 2, WROWS, WCOLS), np.float32)
        ytab = np.zeros((2, 2, 2, NY + 1, 128), np.float32)
        rowofp = np.zeros((2, 2, 2, 128), np.float32)
        yshm = np.zeros((2, 2, 2, 128, TCOL), np.float32)
        xjm = np.zeros((2, 2, 2, 128, TCOL), np.float32)
        jabs = np.zeros((2, 128, TCOL), np.float32)
        cxm = np.zeros((2, 128, TCOL), np.float32)
        cym = np.zeros((2, 128), np.float32)
        ryv = np.zeros((2, 3, 2, 128), np.float32)
        rowv = np.zeros((2, 128), np.float32)
        zmm = np.zeros((2, 128), np.float32)
        sc = np.zeros((32, 128), np.float32)
        sc[0, :] = (f / 2.0) ** 2
        for ti in range(2):
            for i in range(3):
                sc[1 + ti * 3 + i, :] = bs[ti][i]
                sc[7 + ti * 3 + i, :] = Ms[ti][i, 0]
        hx, hy = (W_IMG + 1) / 2.0, (H_IMG + 1) / 2.0
        cxo, cyo = (W_IMG - 1) / 2.0, (H_IMG - 1) / 2.0
        sc[13, :] = EPS
        sc[14, :] = 0.0          # additive zero (Square bias)
        sc[15, :] = 1e-12        # tiny for sqrt(n2)
        sc[16, :] = 1.0 / hx
        sc[17, :] = -cxo / hx
        sc[18, :] = 1.0 / hy
        sc[19, :] = -cyo / hy
        sc[20, :] = 0.0          # relu(pxr) bias
        sc[21, :] = -1.0         # relu(pxr - 1) bias
        sc[22, :] = 1.0          # relu(1 - u1) bias

        for cc in range(2):
            X = C0S[cc] - 6 + np.arange(TCOL, dtype=np.float64)
            jabs[cc, :, :] = X[None, :].astype(np.float32)
            cxm[cc, :, :] = (cx - X)[None, :].astype(np.float32)
        for rc in range(2):
            Yrows = r_lo + R0S[rc] - 6 + np.arange(128, dtype=np.float64)
            cym[rc, :] = (cy - Yrows).astype(np.float32)
            rowv[rc, :] = ((Yrows >= 0) & (Yrows < H_IMG)).astype(np.float32)
            p = np.arange(128)
            lo, hi = (6, 122) if rc == 0 else (12, 76)
            shard_ok = (Yrows >= r_lo) & (Yrows < r_lo + SHARD)
            zmm[rc, :] = ((p >= lo) & (p < hi) & shard_ok).astype(np.float32)
            for ti in range(2):
                for i in range(3):
                    ryv[ti, i, rc, :] = (Ms[ti][i, 1] * Yrows + Ms[ti][i, 2]).astype(np.float32)

        for ti in range(2):
            M, b = Ms[ti], bs[ti]
            # typical (bulk) jitter offset from a mid-range s
            _, jy_tail = [u - v for u, v in zip(
                smooth(M, b, cx, cy, smin_b), smooth(M, b, cx, cy, 0.0))]
            for rc in range(2):
                Yrows = r_lo + R0S[rc] - 6 + np.arange(WROWS, dtype=np.float64)
                for cc in range(2):
                    X = C0S[cc] - 6 + np.arange(TCOL, dtype=np.float64)
                    xc_mid = X[TCOL // 2]
                    yc_mid = Yrows[64]
                    # column shear (y): per-BLK block
                    wy_c = smooth(M, b, X, yc_mid)[1]
                    wy_mid = smooth(M, b, xc_mid, yc_mid)[1]
                    nblk = (WCOLS + BLK - 1) // BLK
                    yshb = np.zeros(nblk)
                    for bi in range(nblk):
                        c0b = min(bi * BLK + BLK // 2, TCOL - 1)
                        yshb[bi] = np.round(wy_c[c0b] - wy_mid)
                    yshm_row = yshb[(np.arange(TCOL) // BLK)]
                    yshm[ti, rc, cc, :, :] = yshm_row[None, :].astype(np.float32)
                    # row map: rowOf(i) = round(wy_smooth(Y(i), xc_mid)) + bias
                    wy_i = smooth(M, b, xc_mid, Yrows)[1]
                    bias = -1.0 if jy_tail < 0 else 0.0
                    rowOf = np.round(wy_i + bias - 1.0)
                    rowofp[ti, rc, cc, :] = rowOf[0:128].astype(np.float32)
                    for d in range(NY + 1):
                        idx = np.arange(128) + d
                        ytab[ti, rc, cc, d, :] = (rowOf[idx] - rowOf[0:128]).astype(np.float32)
                    # x shear baked per block at row center: colOf(c)
                    wx_c = smooth(M, b, X, yc_mid)[0]
                    xsh = np.round(wx_c - X)  # smooth disp per col
                    xshb = np.zeros(nblk)
                    for bi in range(nblk):
                        c0b = min(bi * BLK + BLK // 2, TCOL - 1)
                        xshb[bi] = xsh[c0b]
                    colsh = xshb[(np.arange(WCOLS) // BLK)]
                    colOf = (np.arange(WCOLS) - JW + X[0] + colsh).astype(np.int64)
                    xjm[ti, rc, cc, :, :] = colOf[JW:JW + TCOL][None, :].astype(np.float32) - 1.0
                    # window content
                    rows_i = rowOf.astype(np.int64)
                    ysh_cols = yshm_row.astype(np.int64)
                    # win[i, c] = Tpad[rowOf(i) + ysh(c) + PY, colOf(c) + PX]
                    ri = np.clip(rows_i[:, None] + np.concatenate([ysh_cols, np.full(WCOLS - TCOL, ysh_cols[-1])])[None, :WCOLS].astype(np.int64) + PY, 0, H_IMG + 2 * PY - 1)
                    ci = np.clip(colOf[None, :] + PX, 0, W_IMG + 2 * PX - 1)
                    win[ti, rc, cc] = Tpad[ti][ri, np.broadcast_to(ci, ri.shape)]

        band7 = np.zeros((128, 128), np.float32)
        for i_ in range(128):
            for j_ in range(max(0, i_ - 3), min(128, i_ + 4)):
                band7[i_, j_] = 1.0
        in_maps.append({
            "depth_band": depth_band,
            "ref_band": ref_band.astype(ml_dtypes.bfloat16),
            "win": win.astype(ml_dtypes.bfloat16),
            "band7": band7.astype(ml_dtypes.bfloat16),
            "rowofp": rowofp,
            "ytab": ytab, "yshm": yshm, "xjm": xjm, "jabs": jabs,
            "cxm": cxm, "cym": cym, "sc": sc, "ryv": ryv,
            "rowv": rowv, "zmm": zmm,
        })
    return in_maps


def kernel(focal, axis_angles, centers, ref_image, ref_depthmap, target_images):
    from concourse.bass_utils import run_bass_kernel_spmd

    focal = np.asarray(focal, np.float32)
    axis_angles = np.asarray(axis_angles, np.float32)
    centers = np.asarray(centers, np.float32)
    ref_image = np.asarray(ref_image, np.float32)
    ref_depthmap = np.asarray(ref_depthmap, np.float32)
    target_images = np.asarray(target_images, np.float32)

    if "nc" not in _cache:
        _cache["nc"] = _build_program()
    nc = _cache["nc"]

    in_maps = _host_prep(focal, axis_angles, centers, ref_image,
                         ref_depthmap, target_images)
    res = run_bass_kernel_spmd(nc, in_maps, list(range(NCORES)))
    _cache["last_results"] = res

    total_score = np.float32(0.0)
    total_pixels = np.float32(0.0)
    for k in range(NCORES):
        a = res.results[k]["acc"]
        total_score += a[:, 0].sum(dtype=np.float32) + a[:, 1].sum(dtype=np.float32)
        total_pixels += a[:, 2].sum(dtype=np.float32) + a[:, 3].sum(dtype=np.float32)
    mean_zncc = total_score / max(total_pixels, np.float32(1.0))
    loss = np.float32(0.5) * (np.float32(1.0) - mean_zncc) if total_pixels > 0 else np.float32(0.0)
    return np.float32(loss)


# revision 31
# speedup vs baseline: 2.0536x; 1.4945x over previous
"""Trainium2 Bass kernel for the plane-sweep ZNCC photometric loss module.

Contract: kernel(**inputs) takes the FULL unsharded inputs and returns the
full output (a scalar float32 loss).  Internally the (H, W) pixel grid is
sharded across 8 NeuronCores along H (180 rows per core).  Each core
computes surface normals, the per-pixel homography warp, bilinear sampling
of the target images (via a window-gather scheme), windowed 7x7 ZNCC, and
partial (score, count) sums; the host adds the 16 partial scalars and
finishes the loss.

v2: 7x7 box filters run entirely on the tensor engine (7 shifted matmuls
accumulating in PSUM against a banded 0/1 matrix), the ZNCC chain runs in
bf16, reciprocals use the fast DVE approximation, and elementwise work is
spread across the Vector/Scalar/GpSimd engines.
"""

import numpy as np
import ml_dtypes

H_IMG, W_IMG = 1440, 1920
NCORES = 8
SHARD = H_IMG // NCORES          # 180 rows per core

# chunk geometry (per core): 2 row-chunks x 2 col-chunks
R0S = [0, 110]                   # tile-row offsets inside the shard band
C0S = [0, 960]                   # absolute col origins
TCOL = 972                       # tile cols = 960 + 12 halo
NY = 3                           # y-combo count (window row candidates)
WROWS = 144                      # window rows per chunk (128 + NY + margins)
WCOLS = 980                      # window cols (972 + x-margin)
JW = 3                           # window col of output col q at dx=0
PY, PX = 160, 96                 # zero padding around targets
BLK = 8                          # shear block (cols)
EPS = 1e-8

_cache = {}


def _rodrigues(v):
    th = np.linalg.norm(v) + 1e-12
    k = v / th
    Kx = np.array([[0, -k[2], k[1]], [k[2], 0, -k[0]], [-k[1], k[0], 0]])
    return np.eye(3) + np.sin(th) * Kx + (1 - np.cos(th)) * (Kx @ Kx)


def _build_program():
    """Build the (input-independent) SPMD bass program."""
    import concourse.bacc as bacc
    import concourse.mybir as mybir
    from concourse import tile

    f32 = mybir.dt.float32
    bf16 = mybir.dt.bfloat16
    AL = mybir.AluOpType
    ACT = mybir.ActivationFunctionType

    nc = bacc.Bacc(None, target_bir_lowering=False)

    # ---- DRAM inputs (per core) ----
    d_depth = nc.dram_tensor("depth_band", [240, 1936], f32, kind="ExternalInput")
    d_ref = nc.dram_tensor("ref_band", [238, 1932], bf16, kind="ExternalInput")
    d_win = nc.dram_tensor("win", [2, 2, 2, WROWS, WCOLS], bf16, kind="ExternalInput")
    d_dwin = nc.dram_tensor("dwin", [2, 2, 2, WROWS, WCOLS], bf16, kind="ExternalInput")
    d_ytab = nc.dram_tensor("ytab", [2, 2, 2, NY + 1, 128], f32, kind="ExternalInput")
    d_yshm = nc.dram_tensor("yshm", [2, 2, 2, 128, TCOL], f32, kind="ExternalInput")
    d_xjm = nc.dram_tensor("xjm", [2, 2, 2, 128, TCOL], f32, kind="ExternalInput")
    d_jabs = nc.dram_tensor("jabs", [2, 128, TCOL], f32, kind="ExternalInput")
    d_cxm = nc.dram_tensor("cxm", [2, 128, TCOL], f32, kind="ExternalInput")
    d_cym = nc.dram_tensor("cym", [2, 128], f32, kind="ExternalInput")
    d_sc = nc.dram_tensor("sc", [32, 128], f32, kind="ExternalInput")
    # sc rows: 0: fsq=(f/2)^2, 1..6: b[t][i] (t-major), 7..12: M[t][i,0],
    # 13: EPS, 14: 0.0, 15: 1e-12, 16: 1/hx,
    # 17: -cx/hx, 18: 1/hy, 19: -cy/hy, 20: 0.0, 21: -1.0, 22: +1.0
    d_ryv = nc.dram_tensor("ryv", [2, 3, 2, 128], f32, kind="ExternalInput")
    d_rowv = nc.dram_tensor("rowv", [2, 128], f32, kind="ExternalInput")
    d_zmm = nc.dram_tensor("zmm", [2, 128], f32, kind="ExternalInput")

    d_band = nc.dram_tensor("band7", [128, 128], bf16, kind="ExternalInput")
    d_rowofp = nc.dram_tensor("rowofp", [2, 2, 2, 128], f32, kind="ExternalInput")

    d_acc = nc.dram_tensor("acc", [128, 4], f32, kind="ExternalOutput")

    with tile.TileContext(nc) as tc:
        with tc.tile_pool(name="io", bufs=1) as iop, \
             tc.tile_pool(name="wk", bufs=1) as wk, \
             tc.tile_pool(name="gat", bufs=2) as gp, \
             tc.tile_pool(name="ps", bufs=2, space="PSUM") as psp:

            acc = iop.tile([128, 4], f32)
            nc.vector.memset(acc[:], 0.0)

            jabs_t = [iop.tile([128, TCOL], f32, tag=f"jabs{c}", name=f"jabs{c}") for c in range(2)]
            cxm_t = [iop.tile([128, TCOL], f32, tag=f"cxm{c}", name=f"cxm{c}") for c in range(2)]
            for c in range(2):
                nc.sync.dma_start(jabs_t[c][:], d_jabs[c])
                nc.sync.dma_start(cxm_t[c][:], d_cxm[c])
            cym_t = iop.tile([128, 2], f32)
            nc.sync.dma_start(cym_t[:], d_cym.rearrange("r p -> p r"))
            sc_t = iop.tile([128, 32], f32)
            nc.sync.dma_start(sc_t[:], d_sc.rearrange("r p -> p r"))
            ryv_t = iop.tile([128, 12], f32)
            nc.sync.dma_start(ryv_t[:], d_ryv.rearrange("t i r p -> p (t i r)"))
            rowv_t = iop.tile([128, 2], f32)
            nc.sync.dma_start(rowv_t[:], d_rowv.rearrange("r p -> p r"))
            zmm_t = iop.tile([128, 2], f32)
            nc.sync.dma_start(zmm_t[:], d_zmm.rearrange("r p -> p r"))
            band_t = iop.tile([128, 128], bf16)
            nc.sync.dma_start(band_t[:], d_band[:])

            def box7(out, src, tagp):
                """7x7 box mean of src -> out, valid cols [3, 969) only.
                Pure PE: 7 shifted matmuls accumulate in PSUM; ScalarE
                scales 1/49 out of PSUM."""
                for lo, hi in ((3, 486), (486, 969)):
                    pt = psp.tile([128, 483], f32, tag="bx_ps", name="bx_ps")
                    for dx in range(7):
                        nc.tensor.matmul(pt[:, 0:hi - lo], lhsT=band_t[:],
                                         rhs=src[:, lo - 3 + dx:hi - 3 + dx],
                                         start=(dx == 0), stop=(dx == 6))
                    nc.scalar.mul(out[:, lo:hi], pt[:, 0:hi - lo], 1.0 / 49.0)

            for rc in range(2):
                for cc in range(2):
                    R0 = R0S[rc]
                    # ---------- load depth/ref ----------
                    dptA = wk.tile([128, 974], f32, tag="dptA", name="dptA", bufs=2)
                    nc.sync.dma_start(dptA[:], d_depth[R0:R0 + 128, C0S[cc]:C0S[cc] + 974])
                    dptB = wk.tile([2, 974], f32, tag="dptB", name="dptB")
                    nc.sync.dma_start(dptB[:], d_depth[R0 + 128:R0 + 130, C0S[cc]:C0S[cc] + 974])
                    dsh = wk.tile([128, 974], f32, tag="dsh", name="dsh")
                    nc.sync.dma_start(dsh[0:126, :], dptA[2:128, :])
                    nc.sync.dma_start(dsh[126:128, :], dptB[0:2, :])
                    rt = wk.tile([128, TCOL], bf16, tag="rt", name="rt", bufs=2)
                    nc.sync.dma_start(rt[:], d_ref[R0:R0 + 128, C0S[cc]:C0S[cc] + TCOL])

                    # ---------- s field (shared by both targets) ----------
                    # du2 = d(X+1)-d(X-1), dv2 = d(Y+1)-d(Y-1)   [128, TCOL]
                    du2 = wk.tile([128, TCOL], f32, tag="du2", name="du2")
                    nc.vector.tensor_tensor(out=du2[:], in0=dptA[:, 2:974], in1=dptA[:, 0:TCOL], op=AL.subtract)
                    dv2 = wk.tile([128, TCOL], f32, tag="dv2", name="dv2")
                    nc.vector.tensor_tensor(out=dv2[:], in0=dsh[:, 1:973], in1=dptA[:, 1:973], op=AL.subtract)
                    dep = dptA[:, 1:973]  # depth at the pixel
                    # nz = 0.5*(cxm*du2 + cym*dv2) - dep
                    t1 = wk.tile([128, TCOL], f32, tag="t1", name="t1")
                    nc.vector.tensor_tensor(out=t1[:], in0=cxm_t[cc][:], in1=du2[:], op=AL.mult)
                    nc.vector.scalar_tensor_tensor(out=t1[:], in0=dv2[:], scalar=cym_t[:, rc:rc + 1], in1=t1[:], op0=AL.mult, op1=AL.add)
                    nz = wk.tile([128, TCOL], f32, tag="nz", name="nz")
                    nc.vector.scalar_tensor_tensor(out=nz[:], in0=t1[:], scalar=0.5, in1=dep, op0=AL.mult, op1=AL.subtract)
                    # n2 = nz^2 + fsq*(du2^2 + dv2^2)   (squares on ScalarE)
                    d2u = wk.tile([128, TCOL], f32, tag="d2u", name="d2u")
                    nc.scalar.activation(d2u[:], du2[:], ACT.Square, bias=sc_t[:, 14:15], scale=1.0)
                    d2v = wk.tile([128, TCOL], f32, tag="d2v", name="d2v")
                    nc.scalar.activation(d2v[:], dv2[:], ACT.Square, bias=sc_t[:, 14:15], scale=1.0)
                    nz2 = wk.tile([128, TCOL], f32, tag="nz2", name="nz2")
                    nc.scalar.activation(nz2[:], nz[:], ACT.Square, bias=sc_t[:, 14:15], scale=1.0)
                    n2 = wk.tile([128, TCOL], f32, tag="n2", name="n2")
                    nc.gpsimd.tensor_tensor(out=n2[:], in0=d2u[:], in1=d2v[:], op=AL.add)
                    nc.vector.scalar_tensor_tensor(out=n2[:], in0=n2[:], scalar=sc_t[:, 0:1], in1=nz2[:], op0=AL.mult, op1=AL.add)
                    # nrm = sqrt(n2 + tiny);  den = nrm * dep;  rec = 1/den
                    nrm = wk.tile([128, TCOL], f32, tag="nrm", name="nrm")
                    nc.scalar.activation(nrm[:], n2[:], ACT.Sqrt, bias=sc_t[:, 15:16], scale=1.0)
                    den = wk.tile([128, TCOL], f32, tag="den", name="den")
                    nc.vector.scalar_tensor_tensor(out=den[:], in0=dep, scalar=EPS, in1=nrm[:], op0=AL.add, op1=AL.mult)
                    nc.vector.reciprocal_approx_fast(out=den[:], in_=den[:])
                    # sfield = (0.25*(du2+dv2) - dep) * rec
                    sfield = wk.tile([128, TCOL], f32, tag="sfield", name="sfield")
                    nc.vector.tensor_tensor(out=sfield[:], in0=du2[:], in1=dv2[:], op=AL.add)
                    nc.vector.scalar_tensor_tensor(out=sfield[:], in0=sfield[:], scalar=0.25, in1=dep, op0=AL.mult, op1=AL.subtract)
                    nc.vector.tensor_tensor(out=sfield[:], in0=sfield[:], in1=den[:], op=AL.mult)

                    # shared ref box terms for this chunk (bf16, PE box)
                    # (slices start at even col offsets so bf16 writes stay packed)
                    boxr = wk.tile([128, TCOL], bf16, tag="boxr", name="boxr")
                    box7(boxr, rt, "br")
                    rcen = wk.tile([128, TCOL], bf16, tag="rcen", name="rcen")
                    nc.vector.tensor_tensor(out=rcen[:, 2:970], in0=rt[:, 2:970], in1=boxr[:, 2:970], op=AL.subtract)
                    rc2 = wk.tile([128, TCOL], bf16, tag="rc2", name="rc2")
                    nc.scalar.activation(rc2[:, 2:970], rcen[:, 2:970], ACT.Square, bias=sc_t[:, 14:15], scale=1.0)
                    nc.gpsimd.memset(rc2[:, 0:3], 0.0)
                    nc.gpsimd.memset(rc2[:, 969:TCOL], 0.0)
                    boxrr = wk.tile([128, TCOL], bf16, tag="boxrr", name="boxrr")
                    box7(boxrr, rc2, "brr")

                    for t in range(2):
                        # ---------- warp ----------
                        ww = []
                        for i in range(3):
                            w_ = wk.tile([128, TCOL], f32, tag=f"w{i}", name=f"w{i}")
                            # w = M[i,0]*jabs + ryv   (ScalarE, per-partition scale+bias)
                            nc.scalar.activation(
                                w_[:], jabs_t[cc][:], ACT.Identity,
                                bias=ryv_t[:, t * 6 + i * 2 + rc:t * 6 + i * 2 + rc + 1],
                                scale=sc_t[:, 7 + t * 3 + i:8 + t * 3 + i])
                            # += b[i]*s
                            nc.vector.scalar_tensor_tensor(
                                out=w_[:], in0=sfield[:],
                                scalar=sc_t[:, 1 + t * 3 + i:2 + t * 3 + i],
                                in1=w_[:], op0=AL.mult, op1=AL.add)
                            ww.append(w_)
                        rec = wk.tile([128, TCOL], f32, tag="rec", name="rec")
                        nc.vector.tensor_scalar(out=rec[:], in0=ww[2][:], scalar1=EPS, scalar2=1e-6, op0=AL.add, op1=AL.max)
                        nc.vector.reciprocal_approx_fast(out=rec[:], in_=rec[:])
                        wx = wk.tile([128, TCOL], f32, tag="wx", name="wx")
                        nc.vector.tensor_tensor(out=wx[:], in0=ww[0][:], in1=rec[:], op=AL.mult)
                        wy = wk.tile([128, TCOL], f32, tag="wy", name="wy")
                        nc.gpsimd.tensor_tensor(out=wy[:], in0=ww[1][:], in1=rec[:], op=AL.mult)

                        # oob: big = max(|wx-cx|/hx, |wy-cy|/hy) ; oob iff big > 1
                        axx = wk.tile([128, TCOL], f32, tag="w0", name="axx")
                        nc.scalar.activation(axx[:], wx[:], ACT.Abs,
                                             bias=sc_t[:, 17:18], scale=sc_t[:, 16:17])
                        ayy = wk.tile([128, TCOL], f32, tag="w1", name="ayy")
                        nc.scalar.activation(ayy[:], wy[:], ACT.Abs,
                                             bias=sc_t[:, 19:20], scale=sc_t[:, 18:19])
                        big = wk.tile([128, TCOL], f32, tag="w2", name="big")
                        nc.vector.tensor_tensor(out=big[:], in0=axx[:], in1=ayy[:], op=AL.max)

                        # ---------- y window coordinate (no floor needed) ----------
                        # g = wy - rofp - yshm; the combined select+lerp weight of
                        # window variant d is the tent  relu(1 - |g - ytab_d|).
                        rofp_t = gp.tile([128, 1], f32, tag="rofp", name="rofp")
                        nc.sync.dma_start(rofp_t[:], d_rowofp[t, rc, cc].rearrange("(p o) -> p o", o=1))
                        yshm_t = gp.tile([128, TCOL], f32, tag="yshm", name="yshm")
                        nc.sync.dma_start(yshm_t[:], d_yshm[t, rc, cc])
                        gty = wk.tile([128, TCOL], f32, tag="ycf", name="gty")
                        nc.vector.scalar_tensor_tensor(out=gty[:], in0=wy[:], scalar=rofp_t[:, 0:1], in1=yshm_t[:], op0=AL.subtract, op1=AL.subtract)

                        # ---------- x fraction / 3-tap weights ----------
                        xjm_t = gp.tile([128, TCOL], f32, tag="xjm", name="xjm")
                        nc.sync.dma_start(xjm_t[:], d_xjm[t, rc, cc])
                        pxr = wk.tile([128, TCOL], f32, tag="pxr", name="pxr")
                        nc.vector.tensor_tensor(out=pxr[:], in0=wx[:], in1=xjm_t[:], op=AL.subtract)
                        # c0 = relu(1 - px); vv = relu(px - 1)
                        c0t = wk.tile([128, TCOL], bf16, tag="c0t", name="c0t")
                        nc.scalar.activation(c0t[:], pxr[:], ACT.Relu, bias=sc_t[:, 22:23], scale=-1.0)
                        vv = wk.tile([128, TCOL], bf16, tag="vv", name="vv")
                        nc.scalar.activation(vv[:], pxr[:], ACT.Relu, bias=sc_t[:, 21:22], scale=1.0)

                        # ---------- gather ----------
                        w0 = gp.tile([128, WCOLS], bf16, tag="w0", name="w0", bufs=2)
                        nc.sync.dma_start(w0[:], d_win[t, rc, cc, 0:128, :])
                        w1 = gp.tile([16, WCOLS], bf16, tag="w1", name="w1", bufs=2)
                        nc.sync.dma_start(w1[:], d_win[t, rc, cc, 128:WROWS, :])
                        dw0 = gp.tile([128, WCOLS], bf16, tag="dw0", name="dw0", bufs=2)
                        nc.sync.dma_start(dw0[:], d_dwin[t, rc, cc, 0:128, :])
                        dw1 = gp.tile([16, WCOLS], bf16, tag="dw1", name="dw1", bufs=2)
                        nc.sync.dma_start(dw1[:], d_dwin[t, rc, cc, 128:WROWS, :])
                        ytab_t = gp.tile([128, NY + 1], f32, tag="ytab", name="ytab")
                        nc.sync.dma_start(ytab_t[:], d_ytab[t, rc, cc].rearrange("d p -> p d"))
                        # ytab_t cols hold NEGATED row offsets (-ytab_d)

                        vars_ = [w0]
                        dvars_ = [dw0]
                        for dlt in range(1, NY + 1):
                            v_ = gp.tile([128, WCOLS], bf16, tag=f"var{dlt}", name=f"var{dlt}", bufs=1)
                            nc.sync.dma_start(v_[0:128 - dlt, :], w0[dlt:128, :])
                            nc.sync.dma_start(v_[128 - dlt:128, :], w1[0:dlt, :])
                            vars_.append(v_)
                            dv_ = gp.tile([128, WCOLS], bf16, tag=f"dvar{dlt}", name=f"dvar{dlt}", bufs=1)
                            nc.sync.dma_start(dv_[0:128 - dlt, :], dw0[dlt:128, :])
                            nc.sync.dma_start(dv_[128 - dlt:128, :], dw1[0:dlt, :])
                            dvars_.append(dv_)

                        accv = wk.tile([128, TCOL], bf16, tag="acc0", name="accv")
                        Abf = wk.tile([128, TCOL], f32, tag="acc1", name="Abf")
                        for dlt in range(NY + 1):
                            W_ = vars_[dlt]
                            DW_ = dvars_[dlt]
                            xv = wk.tile([128, TCOL], bf16, tag="xv", name=f"xv{dlt}", bufs=2)
                            tq = wk.tile([128, TCOL], bf16, tag="tqb", name=f"tq{dlt}", bufs=2)
                            # xv = W1 + c0*(W0-W1) - vv*(W1-W2)
                            #    = W1 + c0*DW[j] - vv*DW[j+1]
                            nc.vector.tensor_tensor(out=xv[:], in0=c0t[:], in1=DW_[:, JW:JW + TCOL], op=AL.mult)
                            nc.vector.tensor_tensor(out=tq[:], in0=vv[:], in1=DW_[:, JW + 1:JW + 1 + TCOL], op=AL.mult)
                            nc.vector.tensor_tensor(out=xv[:], in0=xv[:], in1=tq[:], op=AL.subtract)
                            nc.vector.tensor_tensor(out=xv[:], in0=xv[:], in1=W_[:, JW + 1:JW + 1 + TCOL], op=AL.add)
                            # tent weight: w_d = relu(1 - |g - ytab_d|)  (select + y-lerp fused)
                            ad = wk.tile([128, TCOL], f32, tag="mskb", name=f"ad{dlt}", bufs=2)
                            nc.scalar.activation(ad[:], gty[:], ACT.Abs, bias=ytab_t[:, dlt:dlt + 1], scale=1.0)
                            wd = wk.tile([128, TCOL], bf16, tag="mskb1", name=f"wd{dlt}", bufs=2)
                            nc.scalar.activation(wd[:], ad[:], ACT.Relu, bias=sc_t[:, 22:23], scale=-1.0)
                            if dlt == 0:
                                nc.vector.tensor_tensor(out=accv[:], in0=wd[:], in1=xv[:], op=AL.mult)
                            else:
                                xw = wk.tile([128, TCOL], bf16, tag="tqb", name=f"xw{dlt}", bufs=2)
                                nc.vector.tensor_tensor(out=xw[:], in0=wd[:], in1=xv[:], op=AL.mult)
                                if dlt < NY:
                                    nc.gpsimd.tensor_tensor(out=accv[:], in0=accv[:], in1=xw[:], op=AL.add)
                                else:
                                    nc.vector.tensor_tensor(out=Abf[:], in0=accv[:], in1=xw[:], op=AL.add)

                        # oob -> 0  (big <= 1 keeps), then row-valid scaling
                        Aff = wk.tile([128, TCOL], f32, tag="ycf", name="Aff")
                        nc.vector.scalar_tensor_tensor(out=Aff[:], in0=big[:], scalar=1.0, in1=Abf[:], op0=AL.is_le, op1=AL.mult)
                        Af = wk.tile([128, TCOL], bf16, tag="Af", name="Af")
                        nc.scalar.activation(Af[:], Aff[:], ACT.Copy, bias=0.0, scale=rowv_t[:, rc:rc + 1])
                        # col-edge zeroing (global edges only)
                        if cc == 0:
                            nc.gpsimd.memset(Af[:, 0:6], 0.0)
                        if cc == 1:
                            nc.gpsimd.memset(Af[:, 966:TCOL], 0.0)

                        # ---------- zncc ----------
                        valid = wk.tile([128, TCOL], f32, tag="dsh", name="valid")
                        red0 = wk.tile([128, 1], f32, tag="red0", name="red0")
                        nc.vector.tensor_scalar(out=valid[:, 6:966], in0=Af[:, 6:966], scalar1=0.0, scalar2=None, op0=AL.not_equal)
                        nc.vector.tensor_reduce(out=red0[:], in_=valid[:, 6:966], axis=mybir.AxisListType.X, op=AL.add)
                        boxa = wk.tile([128, TCOL], bf16, tag="boxa", name="boxa")
                        box7(boxa, Af, "ba")
                        tcen = wk.tile([128, TCOL], bf16, tag="du2", name="tcen")
                        nc.vector.tensor_tensor(out=tcen[:, 2:970], in0=Af[:, 2:970], in1=boxa[:, 2:970], op=AL.subtract)
                        prod = wk.tile([128, TCOL], bf16, tag="dv2", name="prod")
                        nc.vector.tensor_tensor(out=prod[:, 2:970], in0=rcen[:, 2:970], in1=tcen[:, 2:970], op=AL.mult)
                        nc.gpsimd.memset(prod[:, 0:3], 0.0)
                        nc.gpsimd.memset(prod[:, 969:TCOL], 0.0)
                        cov = wk.tile([128, TCOL], f32, tag="t1", name="cov")
                        box7(cov, prod, "cv")
                        tsq = wk.tile([128, TCOL], bf16, tag="nz", name="tsq")
                        nc.scalar.activation(tsq[:, 2:970], tcen[:, 2:970], ACT.Square, bias=sc_t[:, 14:15], scale=1.0)
                        nc.gpsimd.memset(tsq[:, 0:3], 0.0)
                        nc.gpsimd.memset(tsq[:, 969:TCOL], 0.0)
                        boxtt = wk.tile([128, TCOL], bf16, tag="n2", name="boxtt")
                        box7(boxtt, tsq, "bt")
                        # denom = sqrt(boxtt*boxrr + eps); zm = cov/denom * valid
                        den3 = wk.tile([128, TCOL], f32, tag="d2u", name="den3")
                        nc.vector.tensor_tensor(out=den3[:, 6:966], in0=boxtt[:, 6:966], in1=boxrr[:, 6:966], op=AL.mult)
                        nc.scalar.activation(den3[:, 6:966], den3[:, 6:966], ACT.Sqrt, bias=sc_t[:, 13:14], scale=1.0)
                        nc.vector.reciprocal_approx_fast(out=den3[:, 6:966], in_=den3[:, 6:966])
                        zs = wk.tile([128, TCOL], f32, tag="d2v", name="zs")
                        nc.vector.tensor_tensor(out=zs[:, 6:966], in0=cov[:, 6:966], in1=den3[:, 6:966], op=AL.mult)
                        zv = wk.tile([128, TCOL], f32, tag="nz2", name="zv")
                        red1 = wk.tile([128, 1], f32, tag="red1", name="red1")
                        nc.vector.tensor_tensor(out=zv[:, 6:966], in0=zs[:, 6:966], in1=valid[:, 6:966], op=AL.mult)
                        nc.vector.tensor_reduce(out=red1[:], in_=zv[:, 6:966], axis=mybir.AxisListType.X, op=AL.add)
                        # zmm row-mask + accumulate
                        nc.vector.tensor_scalar(out=red1[:], in0=red1[:], scalar1=zmm_t[:, rc:rc + 1], scalar2=0.0, op0=AL.mult, op1=AL.add)
                        nc.vector.tensor_tensor(out=acc[:, t:t + 1], in0=acc[:, t:t + 1], in1=red1[:], op=AL.add)
                        nc.vector.tensor_scalar(out=red0[:], in0=red0[:], scalar1=zmm_t[:, rc:rc + 1], scalar2=0.0, op0=AL.mult, op1=AL.add)
                        nc.vector.tensor_tensor(out=acc[:, 2 + t:3 + t], in0=acc[:, 2 + t:3 + t], in1=red0[:], op=AL.add)

            nc.sync.dma_start(d_acc[:], acc[:])
    nc.finalize()
    return nc


def _host_prep(focal, axis_angles, centers, ref_image, ref_depthmap, target_images):
    """Build per-core input maps."""
    f = float(focal)
    cx, cy = W_IMG / 2.0, H_IMG / 2.0
    K = np.array([[f, 0, cx - 0.5], [0, f, cy - 0.5], [0, 0, 1.0]])
    K_inv = np.linalg.inv(K)
    R1 = _rodrigues(np.asarray(axis_angles[0], np.float64))
    C1 = np.asarray(centers[0], np.float64)
    Ms, bs = [], []
    for t in (1, 2):
        Rt = _rodrigues(np.asarray(axis_angles[t], np.float64))
        A = K @ Rt
        Ms.append(A @ R1.T @ K_inv)
        bs.append(A @ (C1 - np.asarray(centers[t], np.float64)))

    # padded targets
    Tpad = np.zeros((2, H_IMG + 2 * PY, W_IMG + 2 * PX), np.float32)
    Tpad[:, PY:PY + H_IMG, PX:PX + W_IMG] = target_images

    dmin = max(float(np.min(ref_depthmap)), 1e-3)
    smin_b = -1.05 / dmin

    def smooth(M, b, xg, yg, sv=0.0):
        w0 = M[0, 0] * xg + M[0, 1] * yg + M[0, 2] + b[0] * sv
        w1 = M[1, 0] * xg + M[1, 1] * yg + M[1, 2] + b[1] * sv
        w2 = M[2, 0] * xg + M[2, 1] * yg + M[2, 2] + b[2] * sv
        return w0 / (w2 + EPS), w1 / (w2 + EPS)

    in_maps = []
    for k in range(NCORES):
        r_lo = k * SHARD
        depth_band = np.zeros((240, 1936), np.float32)
        for i in range(240):
            r = r_lo - 7 + i
            if 0 <= r < H_IMG:
                depth_band[i, 8:8 + W_IMG] = ref_depthmap[r]
        ref_band = np.zeros((238, 1932), np.float32)
        for i in range(238):
            r = r_lo - 6 + i
            if 0 <= r < H_IMG:
                ref_band[i, 6:6 + W_IMG] = ref_image[r]

        win = np.zeros((2, 2, 2, WROWS, WCOLS), np.float32)
        ytab = np.zeros((2, 2, 2, NY + 1, 128), np.float32)
        rowofp = np.zeros((2, 2, 2, 128), np.float32)
        yshm = np.zeros((2, 2, 2, 128, TCOL), np.float32)
        xjm = np.zeros((2, 2, 2, 128, TCOL), np.float32)
        jabs = np.zeros((2, 128, TCOL), np.float32)
        cxm = np.zeros((2, 128, TCOL), np.float32)
        cym = np.zeros((2, 128), np.float32)
        ryv = np.zeros((2, 3, 2, 128), np.float32)
        rowv = np.zeros((2, 128), np.float32)
        zmm = np.zeros((2, 128), np.float32)
        sc = np.zeros((32, 128), np.float32)
        sc[0, :] = (f / 2.0) ** 2
        for ti in range(2):
            for i in range(3):
                sc[1 + ti * 3 + i, :] = bs[ti][i]
                sc[7 + ti * 3 + i, :] = Ms[ti][i, 0]
        hx, hy = (W_IMG + 1) / 2.0, (H_IMG + 1) / 2.0
        cxo, cyo = (W_IMG - 1) / 2.0, (H_IMG - 1) / 2.0
        sc[13, :] = EPS
        sc[14, :] = 0.0          # additive zero (Square bias)
        sc[15, :] = 1e-12        # tiny for sqrt(n2)
        sc[16, :] = 1.0 / hx
        sc[17, :] = -cxo / hx
        sc[18, :] = 1.0 / hy
        sc[19, :] = -cyo / hy
        sc[20, :] = 0.0          # relu(pxr) bias
        sc[21, :] = -1.0         # relu(pxr - 1) bias
        sc[22, :] = 1.0          # relu(1 - u1) bias

        for cc in range(2):
            X = C0S[cc] - 6 + np.arange(TCOL, dtype=np.float64)
            jabs[cc, :, :] = X[None, :].astype(np.float32)
            cxm[cc, :, :] = (cx - X)[None, :].astype(np.float32)
        for rc in range(2):
            Yrows = r_lo + R0S[rc] - 6 + np.arange(128, dtype=np.float64)
            cym[rc, :] = (cy - Yrows).astype(np.float32)
            rowv[rc, :] = ((Yrows >= 0) & (Yrows < H_IMG)).astype(np.float32)
            p = np.arange(128)
            lo, hi = (6, 122) if rc == 0 else (12, 76)
            shard_ok = (Yrows >= r_lo) & (Yrows < r_lo + SHARD)
            zmm[rc, :] = ((p >= lo) & (p < hi) & shard_ok).astype(np.float32)
            for ti in range(2):
                for i in range(3):
                    ryv[ti, i, rc, :] = (Ms[ti][i, 1] * Yrows + Ms[ti][i, 2]).astype(np.float32)

        for ti in range(2):
            M, b = Ms[ti], bs[ti]
            # typical (bulk) jitter offset from a mid-range s
            _, jy_tail = [u - v for u, v in zip(
                smooth(M, b, cx, cy, smin_b), smooth(M, b, cx, cy, 0.0))]
            for rc in range(2):
                Yrows = r_lo + R0S[rc] - 6 + np.arange(WROWS, dtype=np.float64)
                for cc in range(2):
                    X = C0S[cc] - 6 + np.arange(TCOL, dtype=np.float64)
                    xc_mid = X[TCOL // 2]
                    yc_mid = Yrows[64]
                    # column shear (y): per-BLK block
                    wy_c = smooth(M, b, X, yc_mid)[1]
                    wy_mid = smooth(M, b, xc_mid, yc_mid)[1]
                    nblk = (WCOLS + BLK - 1) // BLK
                    yshb = np.zeros(nblk)
                    for bi in range(nblk):
                        c0b = min(bi * BLK + BLK // 2, TCOL - 1)
                        yshb[bi] = np.round(wy_c[c0b] - wy_mid)
                    yshm_row = yshb[(np.arange(TCOL) // BLK)]
                    yshm[ti, rc, cc, :, :] = yshm_row[None, :].astype(np.float32)
                    # row map: rowOf(i) = round(wy_smooth(Y(i), xc_mid)) + bias
                    wy_i = smooth(M, b, xc_mid, Yrows)[1]
                    bias = -1.0 if jy_tail < 0 else 0.0
                    rowOf = np.round(wy_i + bias - 1.0)
                    rowofp[ti, rc, cc, :] = rowOf[0:128].astype(np.float32)
                    for d in range(NY + 1):
                        idx = np.arange(128) + d
                        # negated: device uses it as the Abs-activation bias
                        ytab[ti, rc, cc, d, :] = -(rowOf[idx] - rowOf[0:128]).astype(np.float32)
                    # x shear baked per block at row center: colOf(c)
                    wx_c = smooth(M, b, X, yc_mid)[0]
                    xsh = np.round(wx_c - X)  # smooth disp per col
                    xshb = np.zeros(nblk)
                    for bi in range(nblk):
                        c0b = min(bi * BLK + BLK // 2, TCOL - 1)
                        xshb[bi] = xsh[c0b]
                    colsh = xshb[(np.arange(WCOLS) // BLK)]
                    colOf = (np.arange(WCOLS) - JW + X[0] + colsh).astype(np.int64)
                    xjm[ti, rc, cc, :, :] = colOf[JW:JW + TCOL][None, :].astype(np.float32) - 1.0
                    # window content
                    rows_i = rowOf.astype(np.int64)
                    ysh_cols = yshm_row.astype(np.int64)
                    # win[i, c] = Tpad[rowOf(i) + ysh(c) + PY, colOf(c) + PX]
                    ri = np.clip(rows_i[:, None] + np.concatenate([ysh_cols, np.full(WCOLS - TCOL, ysh_cols[-1])])[None, :WCOLS].astype(np.int64) + PY, 0, H_IMG + 2 * PY - 1)
                    ci = np.clip(colOf[None, :] + PX, 0, W_IMG + 2 * PX - 1)
                    win[ti, rc, cc] = Tpad[ti][ri, np.broadcast_to(ci, ri.shape)]

        band7 = np.zeros((128, 128), np.float32)
        for i_ in range(128):
            for j_ in range(max(0, i_ - 3), min(128, i_ + 4)):
                band7[i_, j_] = 1.0
        dwin = np.zeros_like(win)
        dwin[..., :-1] = win[..., :-1] - win[..., 1:]
        in_maps.append({
            "depth_band": depth_band,
            "ref_band": ref_band.astype(ml_dtypes.bfloat16),
            "win": win.astype(ml_dtypes.bfloat16),
            "dwin": dwin.astype(ml_dtypes.bfloat16),
            "band7": band7.astype(ml_dtypes.bfloat16),
            "rowofp": rowofp,
            "ytab": ytab, "yshm": yshm, "xjm": xjm, "jabs": jabs,
            "cxm": cxm, "cym": cym, "sc": sc, "ryv": ryv,
            "rowv": rowv, "zmm": zmm,
        })
    return in_maps


def kernel(focal, axis_angles, centers, ref_image, ref_depthmap, target_images):
    from concourse.bass_utils import run_bass_kernel_spmd

    focal = np.asarray(focal, np.float32)
    axis_angles = np.asarray(axis_angles, np.float32)
    centers = np.asarray(centers, np.float32)
    ref_image = np.asarray(ref_image, np.float32)
    ref_depthmap = np.asarray(ref_depthmap, np.float32)
    target_images = np.asarray(target_images, np.float32)

    if "nc" not in _cache:
        _cache["nc"] = _build_program()
    nc = _cache["nc"]

    in_maps = _host_prep(focal, axis_angles, centers, ref_image,
                         ref_depthmap, target_images)
    res = run_bass_kernel_spmd(nc, in_maps, list(range(NCORES)))
    _cache["last_results"] = res

    total_score = np.float32(0.0)
    total_pixels = np.float32(0.0)
    for k in range(NCORES):
        a = res.results[k]["acc"]
        total_score += a[:, 0].sum(dtype=np.float32) + a[:, 1].sum(dtype=np.float32)
        total_pixels += a[:, 2].sum(dtype=np.float32) + a[:, 3].sum(dtype=np.float32)
    mean_zncc = total_score / max(total_pixels, np.float32(1.0))
    loss = np.float32(0.5) * (np.float32(1.0) - mean_zncc) if total_pixels > 0 else np.float32(0.0)
    return np.float32(loss)


# revision 35
# speedup vs baseline: 2.2520x; 1.0966x over previous
"""Trainium2 Bass kernel for the plane-sweep ZNCC photometric loss module.

Contract: kernel(**inputs) takes the FULL unsharded inputs and returns the
full output (a scalar float32 loss).  Internally the (H, W) pixel grid is
sharded across 8 NeuronCores along H (180 rows per core).  Each core
computes surface normals, the per-pixel homography warp, bilinear sampling
of the target images (via a window-gather scheme), windowed 7x7 ZNCC, and
partial (score, count) sums; the host adds the 16 partial scalars and
finishes the loss.

v2: 7x7 box filters run entirely on the tensor engine (7 shifted matmuls
accumulating in PSUM against a banded 0/1 matrix), the ZNCC chain runs in
bf16, reciprocals use the fast DVE approximation, and elementwise work is
spread across the Vector/Scalar/GpSimd engines.
"""

import numpy as np
import ml_dtypes

H_IMG, W_IMG = 1440, 1920
NCORES = 8
SHARD = H_IMG // NCORES          # 180 rows per core

# chunk geometry (per core): 2 row-chunks x 2 col-chunks
R0S = [0, 110]                   # tile-row offsets inside the shard band
C0S = [0, 960]                   # absolute col origins
TCOL = 972                       # tile cols = 960 + 12 halo
NY = 3                           # y-combo count (window row candidates)
WROWS = 144                      # window rows per chunk (128 + NY + margins)
WCOLS = 980                      # window cols (972 + x-margin)
JW = 3                           # window col of output col q at dx=0
PY, PX = 160, 96                 # zero padding around targets
BLK = 8                          # shear block (cols)
EPS = 1e-8

_cache = {}


def _rodrigues(v):
    th = np.linalg.norm(v) + 1e-12
    k = v / th
    Kx = np.array([[0, -k[2], k[1]], [k[2], 0, -k[0]], [-k[1], k[0], 0]])
    return np.eye(3) + np.sin(th) * Kx + (1 - np.cos(th)) * (Kx @ Kx)


def _build_program():
    """Build the (input-independent) SPMD bass program."""
    import concourse.bacc as bacc
    import concourse.mybir as mybir
    from concourse import tile

    f32 = mybir.dt.float32
    bf16 = mybir.dt.bfloat16
    AL = mybir.AluOpType
    ACT = mybir.ActivationFunctionType

    nc = bacc.Bacc(None, target_bir_lowering=False)

    # ---- DRAM inputs (per core) ----
    d_depth = nc.dram_tensor("depth_band", [240, 1936], f32, kind="ExternalInput")
    d_ref = nc.dram_tensor("ref_band", [238, 1932], bf16, kind="ExternalInput")
    d_win = nc.dram_tensor("win", [2, 2, 2, WROWS, WCOLS], bf16, kind="ExternalInput")
    d_dwin = nc.dram_tensor("dwin", [2, 2, 2, WROWS, WCOLS], bf16, kind="ExternalInput")
    d_ytab = nc.dram_tensor("ytab", [2, 2, 2, NY + 1, 128], f32, kind="ExternalInput")
    d_yshm = nc.dram_tensor("yshm", [2, 2, 2, 128, TCOL], f32, kind="ExternalInput")
    d_xjm = nc.dram_tensor("xjm", [2, 2, 2, 128, TCOL], f32, kind="ExternalInput")
    d_jabs = nc.dram_tensor("jabs", [2, 128, TCOL], f32, kind="ExternalInput")
    d_cxm = nc.dram_tensor("cxm", [2, 128, TCOL], f32, kind="ExternalInput")
    d_cym = nc.dram_tensor("cym", [2, 128], f32, kind="ExternalInput")
    d_sc = nc.dram_tensor("sc", [32, 128], f32, kind="ExternalInput")
    # sc rows: 0: fsq=(f/2)^2, 1..6: b[t][i] (t-major), 7..12: M[t][i,0],
    # 13: EPS, 14: 0.0, 15: 1e-12, 16: 1/hx,
    # 17: -cx/hx, 18: 1/hy, 19: -cy/hy, 20: 0.0, 21: -1.0, 22: +1.0
    d_ryv = nc.dram_tensor("ryv", [2, 3, 2, 128], f32, kind="ExternalInput")
    d_rowv = nc.dram_tensor("rowv", [2, 128], f32, kind="ExternalInput")
    d_zmm = nc.dram_tensor("zmm", [2, 128], f32, kind="ExternalInput")

    d_band = nc.dram_tensor("band7", [128, 128], bf16, kind="ExternalInput")
    d_rowofp = nc.dram_tensor("rowofp", [2, 2, 2, 128], f32, kind="ExternalInput")

    d_acc = nc.dram_tensor("acc", [128, 4], f32, kind="ExternalOutput")

    with tile.TileContext(nc) as tc:
        with tc.tile_pool(name="io", bufs=1) as iop, \
             tc.tile_pool(name="wk", bufs=1) as wk, \
             tc.tile_pool(name="gat", bufs=2) as gp, \
             tc.tile_pool(name="ps", bufs=2, space="PSUM") as psp:

            acc = iop.tile([128, 4], f32)
            nc.vector.memset(acc[:], 0.0)

            jabs_t = [iop.tile([128, TCOL], f32, tag=f"jabs{c}", name=f"jabs{c}") for c in range(2)]
            cxm_t = [iop.tile([128, TCOL], f32, tag=f"cxm{c}", name=f"cxm{c}") for c in range(2)]
            for c in range(2):
                nc.sync.dma_start(jabs_t[c][:], d_jabs[c])
                nc.sync.dma_start(cxm_t[c][:], d_cxm[c])
            cym_t = iop.tile([128, 2], f32)
            nc.sync.dma_start(cym_t[:], d_cym.rearrange("r p -> p r"))
            sc_t = iop.tile([128, 32], f32)
            nc.sync.dma_start(sc_t[:], d_sc.rearrange("r p -> p r"))
            ryv_t = iop.tile([128, 12], f32)
            nc.sync.dma_start(ryv_t[:], d_ryv.rearrange("t i r p -> p (t i r)"))
            rowv_t = iop.tile([128, 2], f32)
            nc.sync.dma_start(rowv_t[:], d_rowv.rearrange("r p -> p r"))
            zmm_t = iop.tile([128, 2], f32)
            nc.sync.dma_start(zmm_t[:], d_zmm.rearrange("r p -> p r"))
            band_t = iop.tile([128, 128], bf16)
            nc.sync.dma_start(band_t[:], d_band[:])

            def box7(out, src, tagp):
                """7x7 box mean of src -> out, valid cols [3, 969) only.
                Pure PE: 7 shifted matmuls accumulate in PSUM; ScalarE
                scales 1/49 out of PSUM."""
                for lo, hi in ((3, 486), (486, 969)):
                    pt = psp.tile([128, 483], f32, tag="bx_ps", name="bx_ps")
                    for dx in range(7):
                        nc.tensor.matmul(pt[:, 0:hi - lo], lhsT=band_t[:],
                                         rhs=src[:, lo - 3 + dx:hi - 3 + dx],
                                         start=(dx == 0), stop=(dx == 6))
                    nc.scalar.mul(out[:, lo:hi], pt[:, 0:hi - lo], 1.0 / 49.0)

            for rc in range(2):
                for cc in range(2):
                    R0 = R0S[rc]
                    # ---------- load depth/ref ----------
                    dptA = wk.tile([128, 974], f32, tag="dptA", name="dptA", bufs=2)
                    nc.sync.dma_start(dptA[:], d_depth[R0:R0 + 128, C0S[cc]:C0S[cc] + 974])
                    dptB = wk.tile([2, 974], f32, tag="dptB", name="dptB")
                    nc.sync.dma_start(dptB[:], d_depth[R0 + 128:R0 + 130, C0S[cc]:C0S[cc] + 974])
                    dsh = wk.tile([128, 974], f32, tag="dsh", name="dsh")
                    nc.sync.dma_start(dsh[0:126, :], dptA[2:128, :])
                    nc.sync.dma_start(dsh[126:128, :], dptB[0:2, :])
                    rt = wk.tile([128, TCOL], bf16, tag="rt", name="rt", bufs=2)
                    nc.sync.dma_start(rt[:], d_ref[R0:R0 + 128, C0S[cc]:C0S[cc] + TCOL])

                    # ---------- s field (shared by both targets) ----------
                    # du2 = d(X+1)-d(X-1), dv2 = d(Y+1)-d(Y-1)   [128, TCOL]
                    du2 = wk.tile([128, TCOL], f32, tag="du2", name="du2")
                    nc.vector.tensor_tensor(out=du2[:], in0=dptA[:, 2:974], in1=dptA[:, 0:TCOL], op=AL.subtract)
                    dv2 = wk.tile([128, TCOL], f32, tag="dv2", name="dv2")
                    nc.vector.tensor_tensor(out=dv2[:], in0=dsh[:, 1:973], in1=dptA[:, 1:973], op=AL.subtract)
                    dep = dptA[:, 1:973]  # depth at the pixel
                    # nz = 0.5*(cxm*du2 + cym*dv2) - dep
                    t1 = wk.tile([128, TCOL], f32, tag="t1", name="t1")
                    nc.vector.tensor_tensor(out=t1[:], in0=cxm_t[cc][:], in1=du2[:], op=AL.mult)
                    nc.vector.scalar_tensor_tensor(out=t1[:], in0=dv2[:], scalar=cym_t[:, rc:rc + 1], in1=t1[:], op0=AL.mult, op1=AL.add)
                    nz = wk.tile([128, TCOL], f32, tag="nz", name="nz")
                    nc.vector.scalar_tensor_tensor(out=nz[:], in0=t1[:], scalar=0.5, in1=dep, op0=AL.mult, op1=AL.subtract)
                    # n2 = nz^2 + fsq*(du2^2 + dv2^2)   (squares on ScalarE)
                    d2u = wk.tile([128, TCOL], f32, tag="d2u", name="d2u")
                    nc.scalar.activation(d2u[:], du2[:], ACT.Square, bias=sc_t[:, 14:15], scale=1.0)
                    d2v = wk.tile([128, TCOL], f32, tag="d2v", name="d2v")
                    nc.scalar.activation(d2v[:], dv2[:], ACT.Square, bias=sc_t[:, 14:15], scale=1.0)
                    nz2 = wk.tile([128, TCOL], f32, tag="nz2", name="nz2")
                    nc.scalar.activation(nz2[:], nz[:], ACT.Square, bias=sc_t[:, 14:15], scale=1.0)
                    n2 = wk.tile([128, TCOL], f32, tag="n2", name="n2")
                    nc.vector.tensor_tensor(out=n2[:], in0=d2u[:], in1=d2v[:], op=AL.add)
                    nc.vector.scalar_tensor_tensor(out=n2[:], in0=n2[:], scalar=sc_t[:, 0:1], in1=nz2[:], op0=AL.mult, op1=AL.add)
                    # nrm = sqrt(n2 + tiny);  den = nrm * dep;  rec = 1/den
                    nrm = wk.tile([128, TCOL], f32, tag="nrm", name="nrm")
                    nc.scalar.activation(nrm[:], n2[:], ACT.Sqrt, bias=sc_t[:, 15:16], scale=1.0)
                    den = wk.tile([128, TCOL], f32, tag="den", name="den")
                    nc.vector.scalar_tensor_tensor(out=den[:], in0=dep, scalar=EPS, in1=nrm[:], op0=AL.add, op1=AL.mult)
                    nc.vector.reciprocal_approx_fast(out=den[:], in_=den[:])
                    # sfield = (0.25*(du2+dv2) - dep) * rec
                    sfield = wk.tile([128, TCOL], f32, tag="sfield", name="sfield")
                    nc.vector.tensor_tensor(out=sfield[:], in0=du2[:], in1=dv2[:], op=AL.add)
                    nc.vector.scalar_tensor_tensor(out=sfield[:], in0=sfield[:], scalar=0.25, in1=dep, op0=AL.mult, op1=AL.subtract)
                    nc.vector.tensor_tensor(out=sfield[:], in0=sfield[:], in1=den[:], op=AL.mult)

                    # shared ref box terms for this chunk (bf16, PE box)
                    # (slices start at even col offsets so bf16 writes stay packed)
                    boxr = wk.tile([128, TCOL], bf16, tag="boxr", name="boxr")
                    box7(boxr, rt, "br")
                    rcen = wk.tile([128, TCOL], bf16, tag="rcen", name="rcen")
                    nc.vector.tensor_tensor(out=rcen[:, 2:970], in0=rt[:, 2:970], in1=boxr[:, 2:970], op=AL.subtract)
                    rc2 = wk.tile([128, TCOL], bf16, tag="rc2", name="rc2")
                    nc.scalar.activation(rc2[:, 2:970], rcen[:, 2:970], ACT.Square, bias=sc_t[:, 14:15], scale=1.0)
                    nc.gpsimd.memset(rc2[:, 0:3], 0.0)
                    nc.gpsimd.memset(rc2[:, 969:TCOL], 0.0)
                    boxrr = wk.tile([128, TCOL], bf16, tag="boxrr", name="boxrr")
                    box7(boxrr, rc2, "brr")

                    for t in range(2):
                        # ---------- warp ----------
                        ww = []
                        for i in range(3):
                            w_ = wk.tile([128, TCOL], f32, tag=f"w{i}", name=f"w{i}")
                            # w = M[i,0]*jabs + ryv   (ScalarE, per-partition scale+bias)
                            nc.scalar.activation(
                                w_[:], jabs_t[cc][:], ACT.Identity,
                                bias=ryv_t[:, t * 6 + i * 2 + rc:t * 6 + i * 2 + rc + 1],
                                scale=sc_t[:, 7 + t * 3 + i:8 + t * 3 + i])
                            # += b[i]*s
                            nc.vector.scalar_tensor_tensor(
                                out=w_[:], in0=sfield[:],
                                scalar=sc_t[:, 1 + t * 3 + i:2 + t * 3 + i],
                                in1=w_[:], op0=AL.mult, op1=AL.add)
                            ww.append(w_)
                        rec = wk.tile([128, TCOL], f32, tag="rec", name="rec")
                        nc.vector.tensor_scalar(out=rec[:], in0=ww[2][:], scalar1=EPS, scalar2=1e-6, op0=AL.add, op1=AL.max)
                        nc.vector.reciprocal_approx_fast(out=rec[:], in_=rec[:])
                        wx = wk.tile([128, TCOL], f32, tag="wx", name="wx")
                        nc.vector.tensor_tensor(out=wx[:], in0=ww[0][:], in1=rec[:], op=AL.mult)
                        wy = wk.tile([128, TCOL], f32, tag="wy", name="wy")
                        nc.vector.tensor_tensor(out=wy[:], in0=ww[1][:], in1=rec[:], op=AL.mult)

                        # x-oob: big = |wx-cx|/hx ; oob iff big > 1.  (y-oob needs no
                        # mask: the zero-padded window + tent weights produce the
                        # reference's zero-padded bilinear values exactly.)
                        big = wk.tile([128, TCOL], f32, tag="w0", name="big")
                        nc.scalar.activation(big[:], wx[:], ACT.Abs,
                                             bias=sc_t[:, 17:18], scale=sc_t[:, 16:17])

                        # ---------- y window coordinate (no floor needed) ----------
                        # g = wy - rofp - yshm; the combined select+lerp weight of
                        # window variant d is the tent  relu(1 - |g - ytab_d|).
                        rofp_t = gp.tile([128, 1], f32, tag="rofp", name="rofp")
                        nc.sync.dma_start(rofp_t[:], d_rowofp[t, rc, cc].rearrange("(p o) -> p o", o=1))
                        yshm_t = gp.tile([128, TCOL], f32, tag="yshm", name="yshm")
                        nc.sync.dma_start(yshm_t[:], d_yshm[t, rc, cc])
                        gty = wk.tile([128, TCOL], f32, tag="ycf", name="gty")
                        nc.vector.scalar_tensor_tensor(out=gty[:], in0=wy[:], scalar=rofp_t[:, 0:1], in1=yshm_t[:], op0=AL.subtract, op1=AL.subtract)

                        # ---------- x fraction / 3-tap weights ----------
                        xjm_t = gp.tile([128, TCOL], f32, tag="xjm", name="xjm")
                        nc.sync.dma_start(xjm_t[:], d_xjm[t, rc, cc])
                        pxr = wk.tile([128, TCOL], f32, tag="pxr", name="pxr")
                        nc.vector.tensor_tensor(out=pxr[:], in0=wx[:], in1=xjm_t[:], op=AL.subtract)
                        # c0 = relu(1 - px); vv = relu(px - 1)
                        c0t = wk.tile([128, TCOL], bf16, tag="c0t", name="c0t")
                        nc.scalar.activation(c0t[:], pxr[:], ACT.Relu, bias=sc_t[:, 22:23], scale=-1.0)
                        vv = wk.tile([128, TCOL], bf16, tag="vv", name="vv")
                        nc.scalar.activation(vv[:], pxr[:], ACT.Relu, bias=sc_t[:, 21:22], scale=1.0)

                        # ---------- gather ----------
                        # window variants loaded straight from DRAM (row-shifted views)
                        ytab_t = gp.tile([128, NY + 1], f32, tag="ytab", name="ytab")
                        nc.sync.dma_start(ytab_t[:], d_ytab[t, rc, cc].rearrange("d p -> p d"))
                        # ytab_t cols hold NEGATED row offsets (-ytab_d)

                        vars_ = []
                        dvars_ = []
                        for dlt in range(NY + 1):
                            v_ = gp.tile([128, WCOLS], bf16, tag=f"var{dlt}", name=f"var{dlt}", bufs=1)
                            nc.sync.dma_start(v_[:], d_win[t, rc, cc, dlt:128 + dlt, :])
                            vars_.append(v_)
                            dv_ = gp.tile([128, WCOLS], bf16, tag=f"dvar{dlt}", name=f"dvar{dlt}", bufs=1)
                            nc.sync.dma_start(dv_[:], d_dwin[t, rc, cc, dlt:128 + dlt, :])
                            dvars_.append(dv_)

                        accv = wk.tile([128, TCOL], bf16, tag="acc0", name="accv")
                        Abf = wk.tile([128, TCOL], f32, tag="acc1", name="Abf")
                        for dlt in range(NY + 1):
                            W_ = vars_[dlt]
                            DW_ = dvars_[dlt]
                            xv = wk.tile([128, TCOL], bf16, tag="xv", name=f"xv{dlt}", bufs=2)
                            tq = wk.tile([128, TCOL], bf16, tag="tqb", name=f"tq{dlt}", bufs=2)
                            # xv = W1 + c0*(W0-W1) - vv*(W1-W2)
                            #    = W1 + c0*DW[j] - vv*DW[j+1]
                            nc.vector.tensor_tensor(out=xv[:], in0=c0t[:], in1=DW_[:, JW:JW + TCOL], op=AL.mult)
                            nc.vector.tensor_tensor(out=tq[:], in0=vv[:], in1=DW_[:, JW + 1:JW + 1 + TCOL], op=AL.mult)
                            nc.vector.tensor_tensor(out=xv[:], in0=xv[:], in1=tq[:], op=AL.subtract)
                            nc.vector.tensor_tensor(out=xv[:], in0=xv[:], in1=W_[:, JW + 1:JW + 1 + TCOL], op=AL.add)
                            # tent weight: w_d = relu(1 - |g - ytab_d|)  (select + y-lerp fused)
                            ad = wk.tile([128, TCOL], f32, tag="mskb", name=f"ad{dlt}", bufs=2)
                            nc.scalar.activation(ad[:], gty[:], ACT.Abs, bias=ytab_t[:, dlt:dlt + 1], scale=1.0)
                            wd = wk.tile([128, TCOL], bf16, tag="mskb1", name=f"wd{dlt}", bufs=2)
                            nc.scalar.activation(wd[:], ad[:], ACT.Relu, bias=sc_t[:, 22:23], scale=-1.0)
                            if dlt == 0:
                                nc.vector.tensor_tensor(out=accv[:], in0=wd[:], in1=xv[:], op=AL.mult)
                            else:
                                xw = wk.tile([128, TCOL], bf16, tag="tqb", name=f"xw{dlt}", bufs=2)
                                nc.vector.tensor_tensor(out=xw[:], in0=wd[:], in1=xv[:], op=AL.mult)
                                if dlt < NY:
                                    nc.vector.tensor_tensor(out=accv[:], in0=accv[:], in1=xw[:], op=AL.add)
                                else:
                                    nc.vector.tensor_tensor(out=Abf[:], in0=accv[:], in1=xw[:], op=AL.add)

                        # oob -> 0  (big <= 1 keeps), then row-valid scaling
                        Aff = wk.tile([128, TCOL], f32, tag="ycf", name="Aff")
                        nc.vector.scalar_tensor_tensor(out=Aff[:], in0=big[:], scalar=1.0, in1=Abf[:], op0=AL.is_le, op1=AL.mult)
                        Af = wk.tile([128, TCOL], bf16, tag="Af", name="Af")
                        nc.scalar.activation(Af[:], Aff[:], ACT.Copy, bias=0.0, scale=rowv_t[:, rc:rc + 1])
                        # col-edge zeroing (global edges only)
                        if cc == 0:
                            nc.gpsimd.memset(Af[:, 0:6], 0.0)
                        if cc == 1:
                            nc.gpsimd.memset(Af[:, 966:TCOL], 0.0)

                        # ---------- zncc ----------
                        valid = wk.tile([128, TCOL], f32, tag="dsh", name="valid")
                        red0 = wk.tile([128, 1], f32, tag="red0", name="red0")
                        nc.vector.tensor_scalar(out=valid[:, 6:966], in0=Af[:, 6:966], scalar1=0.0, scalar2=None, op0=AL.not_equal)
                        nc.vector.tensor_reduce(out=red0[:], in_=valid[:, 6:966], axis=mybir.AxisListType.X, op=AL.add)
                        boxa = wk.tile([128, TCOL], bf16, tag="boxa", name="boxa")
                        box7(boxa, Af, "ba")
                        tcen = wk.tile([128, TCOL], bf16, tag="du2", name="tcen")
                        nc.vector.tensor_tensor(out=tcen[:, 2:970], in0=Af[:, 2:970], in1=boxa[:, 2:970], op=AL.subtract)
                        prod = wk.tile([128, TCOL], bf16, tag="dv2", name="prod")
                        nc.vector.tensor_tensor(out=prod[:, 2:970], in0=rcen[:, 2:970], in1=tcen[:, 2:970], op=AL.mult)
                        nc.gpsimd.memset(prod[:, 0:3], 0.0)
                        nc.gpsimd.memset(prod[:, 969:TCOL], 0.0)
                        cov = wk.tile([128, TCOL], f32, tag="t1", name="cov")
                        box7(cov, prod, "cv")
                        tsq = wk.tile([128, TCOL], bf16, tag="nz", name="tsq")
                        nc.scalar.activation(tsq[:, 2:970], tcen[:, 2:970], ACT.Square, bias=sc_t[:, 14:15], scale=1.0)
                        nc.gpsimd.memset(tsq[:, 0:3], 0.0)
                        nc.gpsimd.memset(tsq[:, 969:TCOL], 0.0)
                        boxtt = wk.tile([128, TCOL], bf16, tag="n2", name="boxtt")
                        box7(boxtt, tsq, "bt")
                        # denom = sqrt(boxtt*boxrr + eps); zm = cov/denom * valid
                        den3 = wk.tile([128, TCOL], f32, tag="d2u", name="den3")
                        nc.vector.tensor_tensor(out=den3[:, 6:966], in0=boxtt[:, 6:966], in1=boxrr[:, 6:966], op=AL.mult)
                        nc.scalar.activation(den3[:, 6:966], den3[:, 6:966], ACT.Sqrt, bias=sc_t[:, 13:14], scale=1.0)
                        nc.vector.reciprocal_approx_fast(out=den3[:, 6:966], in_=den3[:, 6:966])
                        zs = wk.tile([128, TCOL], f32, tag="d2v", name="zs")
                        nc.vector.tensor_tensor(out=zs[:, 6:966], in0=cov[:, 6:966], in1=den3[:, 6:966], op=AL.mult)
                        zv = wk.tile([128, TCOL], f32, tag="nz2", name="zv")
                        red1 = wk.tile([128, 1], f32, tag="red1", name="red1")
                        nc.vector.tensor_tensor(out=zv[:, 6:966], in0=zs[:, 6:966], in1=valid[:, 6:966], op=AL.mult)
                        nc.vector.tensor_reduce(out=red1[:], in_=zv[:, 6:966], axis=mybir.AxisListType.X, op=AL.add)
                        # zmm row-mask + accumulate
                        nc.vector.tensor_scalar(out=red1[:], in0=red1[:], scalar1=zmm_t[:, rc:rc + 1], scalar2=0.0, op0=AL.mult, op1=AL.add)
                        nc.vector.tensor_tensor(out=acc[:, t:t + 1], in0=acc[:, t:t + 1], in1=red1[:], op=AL.add)
                        nc.vector.tensor_scalar(out=red0[:], in0=red0[:], scalar1=zmm_t[:, rc:rc + 1], scalar2=0.0, op0=AL.mult, op1=AL.add)
                        nc.vector.tensor_tensor(out=acc[:, 2 + t:3 + t], in0=acc[:, 2 + t:3 + t], in1=red0[:], op=AL.add)

            nc.sync.dma_start(d_acc[:], acc[:])
    nc.finalize()
    return nc


def _host_prep(focal, axis_angles, centers, ref_image, ref_depthmap, target_images):
    """Build per-core input maps."""
    f = float(focal)
    cx, cy = W_IMG / 2.0, H_IMG / 2.0
    K = np.array([[f, 0, cx - 0.5], [0, f, cy - 0.5], [0, 0, 1.0]])
    K_inv = np.linalg.inv(K)
    R1 = _rodrigues(np.asarray(axis_angles[0], np.float64))
    C1 = np.asarray(centers[0], np.float64)
    Ms, bs = [], []
    for t in (1, 2):
        Rt = _rodrigues(np.asarray(axis_angles[t], np.float64))
        A = K @ Rt
        Ms.append(A @ R1.T @ K_inv)
        bs.append(A @ (C1 - np.asarray(centers[t], np.float64)))

    # padded targets
    Tpad = np.zeros((2, H_IMG + 2 * PY, W_IMG + 2 * PX), np.float32)
    Tpad[:, PY:PY + H_IMG, PX:PX + W_IMG] = target_images

    dmin = max(float(np.min(ref_depthmap)), 1e-3)
    smin_b = -1.05 / dmin

    def smooth(M, b, xg, yg, sv=0.0):
        w0 = M[0, 0] * xg + M[0, 1] * yg + M[0, 2] + b[0] * sv
        w1 = M[1, 0] * xg + M[1, 1] * yg + M[1, 2] + b[1] * sv
        w2 = M[2, 0] * xg + M[2, 1] * yg + M[2, 2] + b[2] * sv
        return w0 / (w2 + EPS), w1 / (w2 + EPS)

    in_maps = []
    for k in range(NCORES):
        r_lo = k * SHARD
        depth_band = np.zeros((240, 1936), np.float32)
        for i in range(240):
            r = r_lo - 7 + i
            if 0 <= r < H_IMG:
                depth_band[i, 8:8 + W_IMG] = ref_depthmap[r]
        ref_band = np.zeros((238, 1932), np.float32)
        for i in range(238):
            r = r_lo - 6 + i
            if 0 <= r < H_IMG:
                ref_band[i, 6:6 + W_IMG] = ref_image[r]

        win = np.zeros((2, 2, 2, WROWS, WCOLS), np.float32)
        ytab = np.zeros((2, 2, 2, NY + 1, 128), np.float32)
        rowofp = np.zeros((2, 2, 2, 128), np.float32)
        yshm = np.zeros((2, 2, 2, 128, TCOL), np.float32)
        xjm = np.zeros((2, 2, 2, 128, TCOL), np.float32)
        jabs = np.zeros((2, 128, TCOL), np.float32)
        cxm = np.zeros((2, 128, TCOL), np.float32)
        cym = np.zeros((2, 128), np.float32)
        ryv = np.zeros((2, 3, 2, 128), np.float32)
        rowv = np.zeros((2, 128), np.float32)
        zmm = np.zeros((2, 128), np.float32)
        sc = np.zeros((32, 128), np.float32)
        sc[0, :] = (f / 2.0) ** 2
        for ti in range(2):
            for i in range(3):
                sc[1 + ti * 3 + i, :] = bs[ti][i]
                sc[7 + ti * 3 + i, :] = Ms[ti][i, 0]
        hx, hy = (W_IMG + 1) / 2.0, (H_IMG + 1) / 2.0
        cxo, cyo = (W_IMG - 1) / 2.0, (H_IMG - 1) / 2.0
        sc[13, :] = EPS
        sc[14, :] = 0.0          # additive zero (Square bias)
        sc[15, :] = 1e-12        # tiny for sqrt(n2)
        sc[16, :] = 1.0 / hx
        sc[17, :] = -cxo / hx
        sc[18, :] = 1.0 / hy
        sc[19, :] = -cyo / hy
        sc[20, :] = 0.0          # relu(pxr) bias
        sc[21, :] = -1.0         # relu(pxr - 1) bias
        sc[22, :] = 1.0          # relu(1 - u1) bias

        for cc in range(2):
            X = C0S[cc] - 6 + np.arange(TCOL, dtype=np.float64)
            jabs[cc, :, :] = X[None, :].astype(np.float32)
            cxm[cc, :, :] = (cx - X)[None, :].astype(np.float32)
        for rc in range(2):
            Yrows = r_lo + R0S[rc] - 6 + np.arange(128, dtype=np.float64)
            cym[rc, :] = (cy - Yrows).astype(np.float32)
            rowv[rc, :] = ((Yrows >= 0) & (Yrows < H_IMG)).astype(np.float32)
            p = np.arange(128)
            lo, hi = (6, 122) if rc == 0 else (12, 76)
            shard_ok = (Yrows >= r_lo) & (Yrows < r_lo + SHARD)
            zmm[rc, :] = ((p >= lo) & (p < hi) & shard_ok).astype(np.float32)
            for ti in range(2):
                for i in range(3):
                    ryv[ti, i, rc, :] = (Ms[ti][i, 1] * Yrows + Ms[ti][i, 2]).astype(np.float32)

        for ti in range(2):
            M, b = Ms[ti], bs[ti]
            # typical (bulk) jitter offset from a mid-range s
            _, jy_tail = [u - v for u, v in zip(
                smooth(M, b, cx, cy, smin_b), smooth(M, b, cx, cy, 0.0))]
            for rc in range(2):
                Yrows = r_lo + R0S[rc] - 6 + np.arange(WROWS, dtype=np.float64)
                for cc in range(2):
                    X = C0S[cc] - 6 + np.arange(TCOL, dtype=np.float64)
                    xc_mid = X[TCOL // 2]
                    yc_mid = Yrows[64]
                    # column shear (y): per-BLK block
                    wy_c = smooth(M, b, X, yc_mid)[1]
                    wy_mid = smooth(M, b, xc_mid, yc_mid)[1]
                    nblk = (WCOLS + BLK - 1) // BLK
                    yshb = np.zeros(nblk)
                    for bi in range(nblk):
                        c0b = min(bi * BLK + BLK // 2, TCOL - 1)
                        yshb[bi] = np.round(wy_c[c0b] - wy_mid)
                    yshm_row = yshb[(np.arange(TCOL) // BLK)]
                    yshm[ti, rc, cc, :, :] = yshm_row[None, :].astype(np.float32)
                    # row map: rowOf(i) = round(wy_smooth(Y(i), xc_mid)) + bias
                    wy_i = smooth(M, b, xc_mid, Yrows)[1]
                    bias = -1.0 if jy_tail < 0 else 0.0
                    rowOf = np.round(wy_i + bias - 1.0)
                    rowofp[ti, rc, cc, :] = rowOf[0:128].astype(np.float32)
                    for d in range(NY + 1):
                        idx = np.arange(128) + d
                        # negated: device uses it as the Abs-activation bias
                        ytab[ti, rc, cc, d, :] = -(rowOf[idx] - rowOf[0:128]).astype(np.float32)
                    # x shear baked per block at row center: colOf(c)
                    wx_c = smooth(M, b, X, yc_mid)[0]
                    xsh = np.round(wx_c - X)  # smooth disp per col
                    xshb = np.zeros(nblk)
                    for bi in range(nblk):
                        c0b = min(bi * BLK + BLK // 2, TCOL - 1)
                        xshb[bi] = xsh[c0b]
                    colsh = xshb[(np.arange(WCOLS) // BLK)]
                    colOf = (np.arange(WCOLS) - JW + X[0] + colsh).astype(np.int64)
                    xjm[ti, rc, cc, :, :] = colOf[JW:JW + TCOL][None, :].astype(np.float32) - 1.0
                    # window content
                    rows_i = rowOf.astype(np.int64)
                    ysh_cols = yshm_row.astype(np.int64)
                    # win[i, c] = Tpad[rowOf(i) + ysh(c) + PY, colOf(c) + PX]
                    ri = np.clip(rows_i[:, None] + np.concatenate([ysh_cols, np.full(WCOLS - TCOL, ysh_cols[-1])])[None, :WCOLS].astype(np.int64) + PY, 0, H_IMG + 2 * PY - 1)
                    ci = np.clip(colOf[None, :] + PX, 0, W_IMG + 2 * PX - 1)
                    win[ti, rc, cc] = Tpad[ti][ri, np.broadcast_to(ci, ri.shape)]

        band7 = np.zeros((128, 128), np.float32)
        for i_ in range(128):
            for j_ in range(max(0, i_ - 3), min(128, i_ + 4)):
                band7[i_, j_] = 1.0
        dwin = np.zeros_like(win)
        dwin[..., :-1] = win[..., :-1] - win[..., 1:]
        in_maps.append({
            "depth_band": depth_band,
            "ref_band": ref_band.astype(ml_dtypes.bfloat16),
            "win": win.astype(ml_dtypes.bfloat16),
            "dwin": dwin.astype(ml_dtypes.bfloat16),
            "band7": band7.astype(ml_dtypes.bfloat16),
            "rowofp": rowofp,
            "ytab": ytab, "yshm": yshm, "xjm": xjm, "jabs": jabs,
            "cxm": cxm, "cym": cym, "sc": sc, "ryv": ryv,
            "rowv": rowv, "zmm": zmm,
        })
    return in_maps


def kernel(focal, axis_angles, centers, ref_image, ref_depthmap, target_images):
    from concourse.bass_utils import run_bass_kernel_spmd

    focal = np.asarray(focal, np.float32)
    axis_angles = np.asarray(axis_angles, np.float32)
    centers = np.asarray(centers, np.float32)
    ref_image = np.asarray(ref_image, np.float32)
    ref_depthmap = np.asarray(ref_depthmap, np.float32)
    target_images = np.asarray(target_images, np.float32)

    if "nc" not in _cache:
        _cache["nc"] = _build_program()
    nc = _cache["nc"]

    in_maps = _host_prep(focal, axis_angles, centers, ref_image,
                         ref_depthmap, target_images)
    res = run_bass_kernel_spmd(nc, in_maps, list(range(NCORES)))
    _cache["last_results"] = res

    total_score = np.float32(0.0)
    total_pixels = np.float32(0.0)
    for k in range(NCORES):
        a = res.results[k]["acc"]
        total_score += a[:, 0].sum(dtype=np.float32) + a[:, 1].sum(dtype=np.float32)
        total_pixels += a[:, 2].sum(dtype=np.float32) + a[:, 3].sum(dtype=np.float32)
    mean_zncc = total_score / max(total_pixels, np.float32(1.0))
    loss = np.float32(0.5) * (np.float32(1.0) - mean_zncc) if total_pixels > 0 else np.float32(0.0)
    return np.float32(loss)


# revision 36
# speedup vs baseline: 2.3092x; 1.0254x over previous
"""Trainium2 Bass kernel for the plane-sweep ZNCC photometric loss module.

Contract: kernel(**inputs) takes the FULL unsharded inputs and returns the
full output (a scalar float32 loss).  Internally the (H, W) pixel grid is
sharded across 8 NeuronCores along H (180 rows per core).  Each core
computes surface normals, the per-pixel homography warp, bilinear sampling
of the target images (via a window-gather scheme), windowed 7x7 ZNCC, and
partial (score, count) sums; the host adds the 16 partial scalars and
finishes the loss.

v2: 7x7 box filters run entirely on the tensor engine (7 shifted matmuls
accumulating in PSUM against a banded 0/1 matrix), the ZNCC chain runs in
bf16, reciprocals use the fast DVE approximation, and elementwise work is
spread across the Vector/Scalar/GpSimd engines.
"""

import numpy as np
import ml_dtypes

H_IMG, W_IMG = 1440, 1920
NCORES = 8
SHARD = H_IMG // NCORES          # 180 rows per core

# chunk geometry (per core): 2 row-chunks x 2 col-chunks
R0S = [0, 110]                   # tile-row offsets inside the shard band
C0S = [0, 960]                   # absolute col origins
TCOL = 972                       # tile cols = 960 + 12 halo
NY = 3                           # y-combo count (window row candidates)
WROWS = 144                      # window rows per chunk (128 + NY + margins)
WCOLS = 980                      # window cols (972 + x-margin)
JW = 3                           # window col of output col q at dx=0
PY, PX = 160, 96                 # zero padding around targets
BLK = 8                          # shear block (cols)
EPS = 1e-8

_cache = {}


def _rodrigues(v):
    th = np.linalg.norm(v) + 1e-12
    k = v / th
    Kx = np.array([[0, -k[2], k[1]], [k[2], 0, -k[0]], [-k[1], k[0], 0]])
    return np.eye(3) + np.sin(th) * Kx + (1 - np.cos(th)) * (Kx @ Kx)


def _build_program():
    """Build the (input-independent) SPMD bass program."""
    import concourse.bacc as bacc
    import concourse.mybir as mybir
    from concourse import tile

    f32 = mybir.dt.float32
    bf16 = mybir.dt.bfloat16
    AL = mybir.AluOpType
    ACT = mybir.ActivationFunctionType

    nc = bacc.Bacc(None, target_bir_lowering=False)

    # ---- DRAM inputs (per core) ----
    d_depth = nc.dram_tensor("depth_band", [240, 1936], f32, kind="ExternalInput")
    d_ref = nc.dram_tensor("ref_band", [238, 1932], bf16, kind="ExternalInput")
    d_win = nc.dram_tensor("win", [2, 2, 2, WROWS, WCOLS], bf16, kind="ExternalInput")
    d_dwin = nc.dram_tensor("dwin", [2, 2, 2, WROWS, WCOLS], bf16, kind="ExternalInput")
    d_ytab = nc.dram_tensor("ytab", [2, 2, 2, NY + 1, 128], f32, kind="ExternalInput")
    d_yshm = nc.dram_tensor("yshm", [2, 2, 2, 128, TCOL], f32, kind="ExternalInput")
    d_xjm = nc.dram_tensor("xjm", [2, 2, 2, 128, TCOL], f32, kind="ExternalInput")
    d_jabs = nc.dram_tensor("jabs", [2, 128, TCOL], f32, kind="ExternalInput")
    d_cxm = nc.dram_tensor("cxm", [2, 128, TCOL], f32, kind="ExternalInput")
    d_cym = nc.dram_tensor("cym", [2, 128], f32, kind="ExternalInput")
    d_sc = nc.dram_tensor("sc", [32, 128], f32, kind="ExternalInput")
    # sc rows: 0: fsq=(f/2)^2, 1..6: b[t][i] (t-major), 7..12: M[t][i,0],
    # 13: EPS, 14: 0.0, 15: 1e-12, 16: 1/hx,
    # 17: -cx/hx, 18: 1/hy, 19: -cy/hy, 20: 0.0, 21: -1.0, 22: +1.0
    d_ryv = nc.dram_tensor("ryv", [2, 3, 2, 128], f32, kind="ExternalInput")
    d_rowv = nc.dram_tensor("rowv", [2, 128], f32, kind="ExternalInput")
    d_zmm = nc.dram_tensor("zmm", [2, 128], f32, kind="ExternalInput")

    d_band = nc.dram_tensor("band7", [128, 128], bf16, kind="ExternalInput")
    d_rowofp = nc.dram_tensor("rowofp", [2, 2, 2, 128], f32, kind="ExternalInput")

    d_acc = nc.dram_tensor("acc", [128, 4], f32, kind="ExternalOutput")

    with tile.TileContext(nc) as tc:
        with tc.tile_pool(name="io", bufs=1) as iop, \
             tc.tile_pool(name="wk", bufs=1) as wk, \
             tc.tile_pool(name="gat", bufs=2) as gp, \
             tc.tile_pool(name="ps", bufs=2, space="PSUM") as psp:

            acc = iop.tile([128, 4], f32)
            nc.vector.memset(acc[:], 0.0)

            jabs_t = [iop.tile([128, TCOL], f32, tag=f"jabs{c}", name=f"jabs{c}") for c in range(2)]
            cxm_t = [iop.tile([128, TCOL], f32, tag=f"cxm{c}", name=f"cxm{c}") for c in range(2)]
            for c in range(2):
                nc.sync.dma_start(jabs_t[c][:], d_jabs[c])
                nc.sync.dma_start(cxm_t[c][:], d_cxm[c])
            cym_t = iop.tile([128, 2], f32)
            nc.sync.dma_start(cym_t[:], d_cym.rearrange("r p -> p r"))
            sc_t = iop.tile([128, 32], f32)
            nc.sync.dma_start(sc_t[:], d_sc.rearrange("r p -> p r"))
            ryv_t = iop.tile([128, 12], f32)
            nc.sync.dma_start(ryv_t[:], d_ryv.rearrange("t i r p -> p (t i r)"))
            rowv_t = iop.tile([128, 2], f32)
            nc.sync.dma_start(rowv_t[:], d_rowv.rearrange("r p -> p r"))
            zmm_t = iop.tile([128, 2], f32)
            nc.sync.dma_start(zmm_t[:], d_zmm.rearrange("r p -> p r"))
            band_t = iop.tile([128, 128], bf16)
            nc.sync.dma_start(band_t[:], d_band[:])

            def box7(out, src, tagp):
                """7x7 box mean of src -> out, valid cols [3, 969) only.
                Pure PE: 7 shifted matmuls accumulate in PSUM; ScalarE
                scales 1/49 out of PSUM."""
                for lo, hi in ((3, 486), (486, 969)):
                    pt = psp.tile([128, 483], f32, tag="bx_ps", name="bx_ps")
                    for dx in range(7):
                        nc.tensor.matmul(pt[:, 0:hi - lo], lhsT=band_t[:],
                                         rhs=src[:, lo - 3 + dx:hi - 3 + dx],
                                         start=(dx == 0), stop=(dx == 6))
                    nc.scalar.mul(out[:, lo:hi], pt[:, 0:hi - lo], 1.0 / 49.0)

            for rc in range(2):
                for cc in range(2):
                    R0 = R0S[rc]
                    # ---------- load depth/ref ----------
                    dptA = wk.tile([128, 974], f32, tag="dptA", name="dptA", bufs=2)
                    nc.sync.dma_start(dptA[:], d_depth[R0:R0 + 128, C0S[cc]:C0S[cc] + 974])
                    dptB = wk.tile([2, 974], f32, tag="dptB", name="dptB")
                    nc.sync.dma_start(dptB[:], d_depth[R0 + 128:R0 + 130, C0S[cc]:C0S[cc] + 974])
                    dsh = wk.tile([128, 974], f32, tag="dsh", name="dsh")
                    nc.sync.dma_start(dsh[0:126, :], dptA[2:128, :])
                    nc.sync.dma_start(dsh[126:128, :], dptB[0:2, :])
                    rt = wk.tile([128, TCOL], bf16, tag="rt", name="rt", bufs=2)
                    nc.sync.dma_start(rt[:], d_ref[R0:R0 + 128, C0S[cc]:C0S[cc] + TCOL])

                    # ---------- s field (shared by both targets) ----------
                    # du2 = d(X+1)-d(X-1), dv2 = d(Y+1)-d(Y-1)   [128, TCOL]
                    du2 = wk.tile([128, TCOL], f32, tag="du2", name="du2")
                    nc.vector.tensor_tensor(out=du2[:], in0=dptA[:, 2:974], in1=dptA[:, 0:TCOL], op=AL.subtract)
                    dv2 = wk.tile([128, TCOL], f32, tag="dv2", name="dv2")
                    nc.vector.tensor_tensor(out=dv2[:], in0=dsh[:, 1:973], in1=dptA[:, 1:973], op=AL.subtract)
                    dep = dptA[:, 1:973]  # depth at the pixel
                    # nz = 0.5*(cxm*du2 + cym*dv2) - dep
                    t1 = wk.tile([128, TCOL], f32, tag="t1", name="t1")
                    nc.vector.tensor_tensor(out=t1[:], in0=cxm_t[cc][:], in1=du2[:], op=AL.mult)
                    nc.vector.scalar_tensor_tensor(out=t1[:], in0=dv2[:], scalar=cym_t[:, rc:rc + 1], in1=t1[:], op0=AL.mult, op1=AL.add)
                    nz = wk.tile([128, TCOL], f32, tag="nz", name="nz")
                    nc.vector.scalar_tensor_tensor(out=nz[:], in0=t1[:], scalar=0.5, in1=dep, op0=AL.mult, op1=AL.subtract)
                    # n2 = nz^2 + fsq*(du2^2 + dv2^2)   (squares on ScalarE)
                    d2u = wk.tile([128, TCOL], f32, tag="d2u", name="d2u")
                    nc.scalar.activation(d2u[:], du2[:], ACT.Square, bias=sc_t[:, 14:15], scale=1.0)
                    d2v = wk.tile([128, TCOL], f32, tag="d2v", name="d2v")
                    nc.scalar.activation(d2v[:], dv2[:], ACT.Square, bias=sc_t[:, 14:15], scale=1.0)
                    nz2 = wk.tile([128, TCOL], f32, tag="nz2", name="nz2")
                    nc.scalar.activation(nz2[:], nz[:], ACT.Square, bias=sc_t[:, 14:15], scale=1.0)
                    n2 = wk.tile([128, TCOL], f32, tag="n2", name="n2")
                    nc.vector.tensor_tensor(out=n2[:], in0=d2u[:], in1=d2v[:], op=AL.add)
                    nc.vector.scalar_tensor_tensor(out=n2[:], in0=n2[:], scalar=sc_t[:, 0:1], in1=nz2[:], op0=AL.mult, op1=AL.add)
                    # nrm = sqrt(n2 + tiny);  den = nrm * dep;  rec = 1/den
                    nrm = wk.tile([128, TCOL], f32, tag="nrm", name="nrm")
                    nc.scalar.activation(nrm[:], n2[:], ACT.Sqrt, bias=sc_t[:, 15:16], scale=1.0)
                    den = wk.tile([128, TCOL], f32, tag="den", name="den")
                    nc.vector.scalar_tensor_tensor(out=den[:], in0=dep, scalar=EPS, in1=nrm[:], op0=AL.add, op1=AL.mult)
                    nc.vector.reciprocal_approx_fast(out=den[:], in_=den[:])
                    # sfield = (0.25*(du2+dv2) - dep) * rec
                    sfield = wk.tile([128, TCOL], f32, tag="sfield", name="sfield")
                    nc.vector.tensor_tensor(out=sfield[:], in0=du2[:], in1=dv2[:], op=AL.add)
                    nc.vector.scalar_tensor_tensor(out=sfield[:], in0=sfield[:], scalar=0.25, in1=dep, op0=AL.mult, op1=AL.subtract)
                    nc.vector.tensor_tensor(out=sfield[:], in0=sfield[:], in1=den[:], op=AL.mult)

                    # shared ref box terms for this chunk (bf16, PE box)
                    # (slices start at even col offsets so bf16 writes stay packed)
                    boxr = wk.tile([128, TCOL], bf16, tag="boxr", name="boxr")
                    box7(boxr, rt, "br")
                    rcen = wk.tile([128, TCOL], bf16, tag="rcen", name="rcen")
                    nc.vector.tensor_tensor(out=rcen[:, 2:970], in0=rt[:, 2:970], in1=boxr[:, 2:970], op=AL.subtract)
                    rc2 = wk.tile([128, TCOL], bf16, tag="rc2", name="rc2")
                    nc.scalar.activation(rc2[:, 2:970], rcen[:, 2:970], ACT.Square, bias=sc_t[:, 14:15], scale=1.0)
                    nc.vector.memset(rc2[:, 0:3], 0.0)
                    nc.vector.memset(rc2[:, 969:TCOL], 0.0)
                    boxrr = wk.tile([128, TCOL], bf16, tag="boxrr", name="boxrr")
                    box7(boxrr, rc2, "brr")

                    for t in range(2):
                        # ---------- warp ----------
                        ww = []
                        for i in range(3):
                            w_ = wk.tile([128, TCOL], f32, tag=f"w{i}", name=f"w{i}")
                            # w = M[i,0]*jabs + ryv   (ScalarE, per-partition scale+bias)
                            nc.scalar.activation(
                                w_[:], jabs_t[cc][:], ACT.Identity,
                                bias=ryv_t[:, t * 6 + i * 2 + rc:t * 6 + i * 2 + rc + 1],
                                scale=sc_t[:, 7 + t * 3 + i:8 + t * 3 + i])
                            # += b[i]*s
                            nc.vector.scalar_tensor_tensor(
                                out=w_[:], in0=sfield[:],
                                scalar=sc_t[:, 1 + t * 3 + i:2 + t * 3 + i],
                                in1=w_[:], op0=AL.mult, op1=AL.add)
                            ww.append(w_)
                        rec = wk.tile([128, TCOL], f32, tag="rec", name="rec")
                        nc.vector.tensor_scalar(out=rec[:], in0=ww[2][:], scalar1=EPS, scalar2=1e-6, op0=AL.add, op1=AL.max)
                        nc.vector.reciprocal_approx_fast(out=rec[:], in_=rec[:])
                        wx = wk.tile([128, TCOL], f32, tag="wx", name="wx")
                        nc.vector.tensor_tensor(out=wx[:], in0=ww[0][:], in1=rec[:], op=AL.mult)
                        wy = wk.tile([128, TCOL], f32, tag="wy", name="wy")
                        nc.vector.tensor_tensor(out=wy[:], in0=ww[1][:], in1=rec[:], op=AL.mult)

                        # x-oob: big = |wx-cx|/hx ; oob iff big > 1.  (y-oob needs no
                        # mask: the zero-padded window + tent weights produce the
                        # reference's zero-padded bilinear values exactly.)
                        big = wk.tile([128, TCOL], f32, tag="w0", name="big")
                        nc.scalar.activation(big[:], wx[:], ACT.Abs,
                                             bias=sc_t[:, 17:18], scale=sc_t[:, 16:17])

                        # ---------- y window coordinate (no floor needed) ----------
                        # g = wy - rofp - yshm; the combined select+lerp weight of
                        # window variant d is the tent  relu(1 - |g - ytab_d|).
                        rofp_t = gp.tile([128, 1], f32, tag="rofp", name="rofp")
                        nc.sync.dma_start(rofp_t[:], d_rowofp[t, rc, cc].rearrange("(p o) -> p o", o=1))
                        yshm_t = gp.tile([128, TCOL], f32, tag="yshm", name="yshm")
                        nc.sync.dma_start(yshm_t[:], d_yshm[t, rc, cc])
                        gty = wk.tile([128, TCOL], f32, tag="ycf", name="gty")
                        nc.vector.scalar_tensor_tensor(out=gty[:], in0=wy[:], scalar=rofp_t[:, 0:1], in1=yshm_t[:], op0=AL.subtract, op1=AL.subtract)

                        # ---------- x fraction / 3-tap weights ----------
                        xjm_t = gp.tile([128, TCOL], f32, tag="xjm", name="xjm")
                        nc.sync.dma_start(xjm_t[:], d_xjm[t, rc, cc])
                        pxr = wk.tile([128, TCOL], f32, tag="pxr", name="pxr")
                        nc.vector.tensor_tensor(out=pxr[:], in0=wx[:], in1=xjm_t[:], op=AL.subtract)
                        # c0 = relu(1 - px); vv = relu(px - 1)
                        c0t = wk.tile([128, TCOL], bf16, tag="c0t", name="c0t")
                        nc.scalar.activation(c0t[:], pxr[:], ACT.Relu, bias=sc_t[:, 22:23], scale=-1.0)
                        vv = wk.tile([128, TCOL], bf16, tag="vv", name="vv")
                        nc.scalar.activation(vv[:], pxr[:], ACT.Relu, bias=sc_t[:, 21:22], scale=1.0)

                        # ---------- gather ----------
                        # window variants loaded straight from DRAM (row-shifted views)
                        ytab_t = gp.tile([128, NY + 1], f32, tag="ytab", name="ytab")
                        nc.sync.dma_start(ytab_t[:], d_ytab[t, rc, cc].rearrange("d p -> p d"))
                        # ytab_t cols hold NEGATED row offsets (-ytab_d)

                        vars_ = []
                        dvars_ = []
                        for dlt in range(NY + 1):
                            v_ = gp.tile([128, WCOLS], bf16, tag=f"var{dlt}", name=f"var{dlt}", bufs=2)
                            nc.sync.dma_start(v_[:], d_win[t, rc, cc, dlt:128 + dlt, :])
                            vars_.append(v_)
                            dv_ = gp.tile([128, WCOLS], bf16, tag=f"dvar{dlt}", name=f"dvar{dlt}", bufs=2)
                            nc.sync.dma_start(dv_[:], d_dwin[t, rc, cc, dlt:128 + dlt, :])
                            dvars_.append(dv_)

                        accv = wk.tile([128, TCOL], bf16, tag="acc0", name="accv")
                        Abf = wk.tile([128, TCOL], f32, tag="acc1", name="Abf")
                        for dlt in range(NY + 1):
                            W_ = vars_[dlt]
                            DW_ = dvars_[dlt]
                            xv = wk.tile([128, TCOL], bf16, tag="xv", name=f"xv{dlt}", bufs=2)
                            tq = wk.tile([128, TCOL], bf16, tag="tqb", name=f"tq{dlt}", bufs=2)
                            # xv = W1 + c0*(W0-W1) - vv*(W1-W2)
                            #    = W1 + c0*DW[j] - vv*DW[j+1]
                            nc.vector.tensor_tensor(out=xv[:], in0=c0t[:], in1=DW_[:, JW:JW + TCOL], op=AL.mult)
                            nc.vector.tensor_tensor(out=tq[:], in0=vv[:], in1=DW_[:, JW + 1:JW + 1 + TCOL], op=AL.mult)
                            nc.vector.tensor_tensor(out=xv[:], in0=xv[:], in1=tq[:], op=AL.subtract)
                            nc.vector.tensor_tensor(out=xv[:], in0=xv[:], in1=W_[:, JW + 1:JW + 1 + TCOL], op=AL.add)
                            # tent weight: w_d = relu(1 - |g - ytab_d|)  (select + y-lerp fused)
                            ad = wk.tile([128, TCOL], f32, tag="mskb", name=f"ad{dlt}", bufs=2)
                            nc.scalar.activation(ad[:], gty[:], ACT.Abs, bias=ytab_t[:, dlt:dlt + 1], scale=1.0)
                            wd = wk.tile([128, TCOL], bf16, tag="mskb1", name=f"wd{dlt}", bufs=2)
                            nc.scalar.activation(wd[:], ad[:], ACT.Relu, bias=sc_t[:, 22:23], scale=-1.0)
                            if dlt == 0:
                                nc.vector.tensor_tensor(out=accv[:], in0=wd[:], in1=xv[:], op=AL.mult)
                            else:
                                xw = wk.tile([128, TCOL], bf16, tag="tqb", name=f"xw{dlt}", bufs=2)
                                nc.vector.tensor_tensor(out=xw[:], in0=wd[:], in1=xv[:], op=AL.mult)
                                if dlt < NY:
                                    nc.vector.tensor_tensor(out=accv[:], in0=accv[:], in1=xw[:], op=AL.add)
                                else:
                                    nc.vector.tensor_tensor(out=Abf[:], in0=accv[:], in1=xw[:], op=AL.add)

                        # oob -> 0  (big <= 1 keeps), then row-valid scaling
                        Aff = wk.tile([128, TCOL], f32, tag="ycf", name="Aff")
                        nc.vector.scalar_tensor_tensor(out=Aff[:], in0=big[:], scalar=1.0, in1=Abf[:], op0=AL.is_le, op1=AL.mult)
                        Af = wk.tile([128, TCOL], bf16, tag="Af", name="Af")
                        nc.scalar.activation(Af[:], Aff[:], ACT.Copy, bias=0.0, scale=rowv_t[:, rc:rc + 1])
                        # col-edge zeroing (global edges only)
                        if cc == 0:
                            nc.vector.memset(Af[:, 0:6], 0.0)
                        if cc == 1:
                            nc.vector.memset(Af[:, 966:TCOL], 0.0)

                        # ---------- zncc ----------
                        valid = wk.tile([128, TCOL], f32, tag="dsh", name="valid")
                        red0 = wk.tile([128, 1], f32, tag="red0", name="red0")
                        nc.vector.tensor_scalar(out=valid[:, 6:966], in0=Af[:, 6:966], scalar1=0.0, scalar2=None, op0=AL.not_equal)
                        nc.vector.tensor_reduce(out=red0[:], in_=valid[:, 6:966], axis=mybir.AxisListType.X, op=AL.add)
                        boxa = wk.tile([128, TCOL], bf16, tag="boxa", name="boxa")
                        box7(boxa, Af, "ba")
                        tcen = wk.tile([128, TCOL], bf16, tag="du2", name="tcen")
                        nc.vector.tensor_tensor(out=tcen[:, 2:970], in0=Af[:, 2:970], in1=boxa[:, 2:970], op=AL.subtract)
                        prod = wk.tile([128, TCOL], bf16, tag="dv2", name="prod")
                        nc.vector.tensor_tensor(out=prod[:, 2:970], in0=rcen[:, 2:970], in1=tcen[:, 2:970], op=AL.mult)
                        nc.vector.memset(prod[:, 0:3], 0.0)
                        nc.vector.memset(prod[:, 969:TCOL], 0.0)
                        cov = wk.tile([128, TCOL], f32, tag="t1", name="cov")
                        box7(cov, prod, "cv")
                        tsq = wk.tile([128, TCOL], bf16, tag="nz", name="tsq")
                        nc.scalar.activation(tsq[:, 2:970], tcen[:, 2:970], ACT.Square, bias=sc_t[:, 14:15], scale=1.0)
                        nc.vector.memset(tsq[:, 0:3], 0.0)
                        nc.vector.memset(tsq[:, 969:TCOL], 0.0)
                        boxtt = wk.tile([128, TCOL], bf16, tag="n2", name="boxtt")
                        box7(boxtt, tsq, "bt")
                        # denom = sqrt(boxtt*boxrr + eps); zm = cov/denom * valid
                        den3 = wk.tile([128, TCOL], bf16, tag="den3b", name="den3")
                        nc.vector.tensor_tensor(out=den3[:, 6:966], in0=boxtt[:, 6:966], in1=boxrr[:, 6:966], op=AL.mult)
                        sq = wk.tile([128, TCOL], f32, tag="d2u", name="sq")
                        nc.scalar.activation(sq[:, 6:966], den3[:, 6:966], ACT.Sqrt, bias=sc_t[:, 13:14], scale=1.0)
                        nc.vector.reciprocal_approx_fast(out=sq[:, 6:966], in_=sq[:, 6:966])
                        zs = wk.tile([128, TCOL], f32, tag="d2v", name="zs")
                        nc.vector.tensor_tensor(out=zs[:, 6:966], in0=cov[:, 6:966], in1=sq[:, 6:966], op=AL.mult)
                        zv = wk.tile([128, TCOL], f32, tag="nz2", name="zv")
                        red1 = wk.tile([128, 1], f32, tag="red1", name="red1")
                        nc.vector.tensor_tensor(out=zv[:, 6:966], in0=zs[:, 6:966], in1=valid[:, 6:966], op=AL.mult)
                        nc.vector.tensor_reduce(out=red1[:], in_=zv[:, 6:966], axis=mybir.AxisListType.X, op=AL.add)
                        # zmm row-mask + accumulate
                        nc.vector.tensor_scalar(out=red1[:], in0=red1[:], scalar1=zmm_t[:, rc:rc + 1], scalar2=0.0, op0=AL.mult, op1=AL.add)
                        nc.vector.tensor_tensor(out=acc[:, t:t + 1], in0=acc[:, t:t + 1], in1=red1[:], op=AL.add)
                        nc.vector.tensor_scalar(out=red0[:], in0=red0[:], scalar1=zmm_t[:, rc:rc + 1], scalar2=0.0, op0=AL.mult, op1=AL.add)
                        nc.vector.tensor_tensor(out=acc[:, 2 + t:3 + t], in0=acc[:, 2 + t:3 + t], in1=red0[:], op=AL.add)

            nc.sync.dma_start(d_acc[:], acc[:])
    nc.finalize()
    return nc


def _host_prep(focal, axis_angles, centers, ref_image, ref_depthmap, target_images):
    """Build per-core input maps."""
    f = float(focal)
    cx, cy = W_IMG / 2.0, H_IMG / 2.0
    K = np.array([[f, 0, cx - 0.5], [0, f, cy - 0.5], [0, 0, 1.0]])
    K_inv = np.linalg.inv(K)
    R1 = _rodrigues(np.asarray(axis_angles[0], np.float64))
    C1 = np.asarray(centers[0], np.float64)
    Ms, bs = [], []
    for t in (1, 2):
        Rt = _rodrigues(np.asarray(axis_angles[t], np.float64))
        A = K @ Rt
        Ms.append(A @ R1.T @ K_inv)
        bs.append(A @ (C1 - np.asarray(centers[t], np.float64)))

    # padded targets
    Tpad = np.zeros((2, H_IMG + 2 * PY, W_IMG + 2 * PX), np.float32)
    Tpad[:, PY:PY + H_IMG, PX:PX + W_IMG] = target_images

    dmin = max(float(np.min(ref_depthmap)), 1e-3)
    smin_b = -1.05 / dmin

    def smooth(M, b, xg, yg, sv=0.0):
        w0 = M[0, 0] * xg + M[0, 1] * yg + M[0, 2] + b[0] * sv
        w1 = M[1, 0] * xg + M[1, 1] * yg + M[1, 2] + b[1] * sv
        w2 = M[2, 0] * xg + M[2, 1] * yg + M[2, 2] + b[2] * sv
        return w0 / (w2 + EPS), w1 / (w2 + EPS)

    in_maps = []
    for k in range(NCORES):
        r_lo = k * SHARD
        depth_band = np.zeros((240, 1936), np.float32)
        for i in range(240):
            r = r_lo - 7 + i
            if 0 <= r < H_IMG:
                depth_band[i, 8:8 + W_IMG] = ref_depthmap[r]
        ref_band = np.zeros((238, 1932), np.float32)
        for i in range(238):
            r = r_lo - 6 + i
            if 0 <= r < H_IMG:
                ref_band[i, 6:6 + W_IMG] = ref_image[r]

        win = np.zeros((2, 2, 2, WROWS, WCOLS), np.float32)
        ytab = np.zeros((2, 2, 2, NY + 1, 128), np.float32)
        rowofp = np.zeros((2, 2, 2, 128), np.float32)
        yshm = np.zeros((2, 2, 2, 128, TCOL), np.float32)
        xjm = np.zeros((2, 2, 2, 128, TCOL), np.float32)
        jabs = np.zeros((2, 128, TCOL), np.float32)
        cxm = np.zeros((2, 128, TCOL), np.float32)
        cym = np.zeros((2, 128), np.float32)
        ryv = np.zeros((2, 3, 2, 128), np.float32)
        rowv = np.zeros((2, 128), np.float32)
        zmm = np.zeros((2, 128), np.float32)
        sc = np.zeros((32, 128), np.float32)
        sc[0, :] = (f / 2.0) ** 2
        for ti in range(2):
            for i in range(3):
                sc[1 + ti * 3 + i, :] = bs[ti][i]
                sc[7 + ti * 3 + i, :] = Ms[ti][i, 0]
        hx, hy = (W_IMG + 1) / 2.0, (H_IMG + 1) / 2.0
        cxo, cyo = (W_IMG - 1) / 2.0, (H_IMG - 1) / 2.0
        sc[13, :] = EPS
        sc[14, :] = 0.0          # additive zero (Square bias)
        sc[15, :] = 1e-12        # tiny for sqrt(n2)
        sc[16, :] = 1.0 / hx
        sc[17, :] = -cxo / hx
        sc[18, :] = 1.0 / hy
        sc[19, :] = -cyo / hy
        sc[20, :] = 0.0          # relu(pxr) bias
        sc[21, :] = -1.0         # relu(pxr - 1) bias
        sc[22, :] = 1.0          # relu(1 - u1) bias

        for cc in range(2):
            X = C0S[cc] - 6 + np.arange(TCOL, dtype=np.float64)
            jabs[cc, :, :] = X[None, :].astype(np.float32)
            cxm[cc, :, :] = (cx - X)[None, :].astype(np.float32)
        for rc in range(2):
            Yrows = r_lo + R0S[rc] - 6 + np.arange(128, dtype=np.float64)
            cym[rc, :] = (cy - Yrows).astype(np.float32)
            rowv[rc, :] = ((Yrows >= 0) & (Yrows < H_IMG)).astype(np.float32)
            p = np.arange(128)
            lo, hi = (6, 122) if rc == 0 else (12, 76)
            shard_ok = (Yrows >= r_lo) & (Yrows < r_lo + SHARD)
            zmm[rc, :] = ((p >= lo) & (p < hi) & shard_ok).astype(np.float32)
            for ti in range(2):
                for i in range(3):
                    ryv[ti, i, rc, :] = (Ms[ti][i, 1] * Yrows + Ms[ti][i, 2]).astype(np.float32)

        for ti in range(2):
            M, b = Ms[ti], bs[ti]
            # typical (bulk) jitter offset from a mid-range s
            _, jy_tail = [u - v for u, v in zip(
                smooth(M, b, cx, cy, smin_b), smooth(M, b, cx, cy, 0.0))]
            for rc in range(2):
                Yrows = r_lo + R0S[rc] - 6 + np.arange(WROWS, dtype=np.float64)
                for cc in range(2):
                    X = C0S[cc] - 6 + np.arange(TCOL, dtype=np.float64)
                    xc_mid = X[TCOL // 2]
                    yc_mid = Yrows[64]
                    # column shear (y): per-BLK block
                    wy_c = smooth(M, b, X, yc_mid)[1]
                    wy_mid = smooth(M, b, xc_mid, yc_mid)[1]
                    nblk = (WCOLS + BLK - 1) // BLK
                    yshb = np.zeros(nblk)
                    for bi in range(nblk):
                        c0b = min(bi * BLK + BLK // 2, TCOL - 1)
                        yshb[bi] = np.round(wy_c[c0b] - wy_mid)
                    yshm_row = yshb[(np.arange(TCOL) // BLK)]
                    yshm[ti, rc, cc, :, :] = yshm_row[None, :].astype(np.float32)
                    # row map: rowOf(i) = round(wy_smooth(Y(i), xc_mid)) + bias
                    wy_i = smooth(M, b, xc_mid, Yrows)[1]
                    bias = -1.0 if jy_tail < 0 else 0.0
                    rowOf = np.round(wy_i + bias - 1.0)
                    rowofp[ti, rc, cc, :] = rowOf[0:128].astype(np.float32)
                    for d in range(NY + 1):
                        idx = np.arange(128) + d
                        # negated: device uses it as the Abs-activation bias
                        ytab[ti, rc, cc, d, :] = -(rowOf[idx] - rowOf[0:128]).astype(np.float32)
                    # x shear baked per block at row center: colOf(c)
                    wx_c = smooth(M, b, X, yc_mid)[0]
                    xsh = np.round(wx_c - X)  # smooth disp per col
                    xshb = np.zeros(nblk)
                    for bi in range(nblk):
                        c0b = min(bi * BLK + BLK // 2, TCOL - 1)
                        xshb[bi] = xsh[c0b]
                    colsh = xshb[(np.arange(WCOLS) // BLK)]
                    colOf = (np.arange(WCOLS) - JW + X[0] + colsh).astype(np.int64)
                    xjm[ti, rc, cc, :, :] = colOf[JW:JW + TCOL][None, :].astype(np.float32) - 1.0
                    # window content
                    rows_i = rowOf.astype(np.int64)
                    ysh_cols = yshm_row.astype(np.int64)
                    # win[i, c] = Tpad[rowOf(i) + ysh(c) + PY, colOf(c) + PX]
                    ri = np.clip(rows_i[:, None] + np.concatenate([ysh_cols, np.full(WCOLS - TCOL, ysh_cols[-1])])[None, :WCOLS].astype(np.int64) + PY, 0, H_IMG + 2 * PY - 1)
                    ci = np.clip(colOf[None, :] + PX, 0, W_IMG + 2 * PX - 1)
                    win[ti, rc, cc] = Tpad[ti][ri, np.broadcast_to(ci, ri.shape)]

        band7 = np.zeros((128, 128), np.float32)
        for i_ in range(128):
            for j_ in range(max(0, i_ - 3), min(128, i_ + 4)):
                band7[i_, j_] = 1.0
        dwin = np.zeros_like(win)
        dwin[..., :-1] = win[..., :-1] - win[..., 1:]
        in_maps.append({
            "depth_band": depth_band,
            "ref_band": ref_band.astype(ml_dtypes.bfloat16),
            "win": win.astype(ml_dtypes.bfloat16),
            "dwin": dwin.astype(ml_dtypes.bfloat16),
            "band7": band7.astype(ml_dtypes.bfloat16),
            "rowofp": rowofp,
            "ytab": ytab, "yshm": yshm, "xjm": xjm, "jabs": jabs,
            "cxm": cxm, "cym": cym, "sc": sc, "ryv": ryv,
            "rowv": rowv, "zmm": zmm,
        })
    return in_maps


def kernel(focal, axis_angles, centers, ref_image, ref_depthmap, target_images):
    from concourse.bass_utils import run_bass_kernel_spmd

    focal = np.asarray(focal, np.float32)
    axis_angles = np.asarray(axis_angles, np.float32)
    centers = np.asarray(centers, np.float32)
    ref_image = np.asarray(ref_image, np.float32)
    ref_depthmap = np.asarray(ref_depthmap, np.float32)
    target_images = np.asarray(target_images, np.float32)

    if "nc" not in _cache:
        _cache["nc"] = _build_program()
    nc = _cache["nc"]

    in_maps = _host_prep(focal, axis_angles, centers, ref_image,
                         ref_depthmap, target_images)
    res = run_bass_kernel_spmd(nc, in_maps, list(range(NCORES)))
    _cache["last_results"] = res

    total_score = np.float32(0.0)
    total_pixels = np.float32(0.0)
    for k in range(NCORES):
        a = res.results[k]["acc"]
        total_score += a[:, 0].sum(dtype=np.float32) + a[:, 1].sum(dtype=np.float32)
        total_pixels += a[:, 2].sum(dtype=np.float32) + a[:, 3].sum(dtype=np.float32)
    mean_zncc = total_score / max(total_pixels, np.float32(1.0))
    loss = np.float32(0.5) * (np.float32(1.0) - mean_zncc) if total_pixels > 0 else np.float32(0.0)
    return np.float32(loss)
